# revision 23
# baseline (speedup 1.0000x reference)
"""Trainium2 Bass kernel for thresholded multi-head attention (v2).

Computes, for x:[b,n,dim] with b=4, n=2048, dim=512, heads=8, dh=64:
    qkv = x @ Wqkv + bqkv ; split q,k,v per head
    dots = q k^T / sqrt(dh) ; attn = softmax(dots)
    attn = where(attn > 0.01, attn, 0) ; out = attn @ v
    return out @ Wout + bout

Sharding over 8 NeuronCores: core c handles batch b = c//2 and head group
g = c%2 (4 of the 8 heads); host sums the two partial output projections
per batch and adds bout.

Numerics: the attention threshold sits within 8.9e-7 (relative) of the
closest entry, so attn must be exact to ~5e-7 near 0.01 or a flip blows the
error budget. S logits are computed from fp16 hi/lo limbs in TWO matmuls:
kh^T qh (64-contraction) plus a stacked [kh;kl]^T [ql;qh] (128-contraction)
covering both cross terms in one PE pass; Z is an exact fp32 elementwise
tree-sum of the eight E-tiles on DVE/GPSIMD, reduced across partitions via
hi/lo fp16 limb matmuls; the attn>0.01 compare is fp32-exact against
c = 0.01*Z via a one-pass custom DVE select. x is transposed and limb-split
host-side. Broadcast matmuls (threshold c, 1/Z) run as fp16 limbs at
1 cyc/row. The PE-side Z/threshold work doubles as queue filler that keeps
the tensor engine's HAM clock gate at full rate.
"""
import os
import sys
import functools

import numpy as np

for _p in ("/opt/trn_rl_repo", "/root/.axon_site", "/root/.axon_site/_ro/trn_rl_repo"):
    if os.path.isdir(_p) and _p not in sys.path:
        sys.path.append(_p)

import ml_dtypes
from contextlib import ExitStack

import concourse.bass as bass
import concourse.bacc as bacc
import concourse.mybir as mybir
import concourse.tile as tile
from concourse import bass_utils

FP32 = mybir.dt.float32
FP16 = mybir.dt.float16
BF16 = mybir.dt.bfloat16
ALU = mybir.AluOpType
AFT = mybir.ActivationFunctionType

# engine for each of the 7 tree adds (kt=1..7):
# v=vector tt, g=gpsimd tt, d=software-DGE DMA with accumulate (runs on the
# DMA engines, nearly free for the compute engines). kt=1 must be v or g.
TREE_ENG = "gdvgvdg"
PV_PAIR = True


def _register_mask_op():
    """One-pass masked keep: out = in0 if in1 < in0 else 0."""
    from concourse.dve_spec import Spec, Src0, Src1, Zero, select
    from concourse import dve_ops as dops

    name = "MASK_KEEP_GT_ANT"
    for op in dops.OPS:
        if op.name == name:
            return op
    op = dops.DveOp(
        name,
        Spec(
            body=select(Src1 < Src0, Src0, Zero),
            reference=lambda in0, in1, s0, s1, imm2: np.where(
                in1 < in0, in0, 0.0).astype(np.float32),
        ),
        subdim=False,
        uops_sha={"v3": "d86f8416d0d7b042", "v4": "f70e64aee8639ca3"},
    )
    dops.OPS.append(op)
    dops._SUB_OPCODE_FOR_NAME[name] = dops._CUSTOM_DVE_ROW_BASE + len(dops.OPS) - 1
    dops.CUSTOM_DVE_SPECS[name] = op.spec
    return op


MASK_OP = _register_mask_op()


def emit_core_kernel(ctx, tc, io, n=2048, dim=512, hc=4, dh=64, qch=512):
    nc = tc.nc
    inner = hc * dh                 # 256
    NT = n // 128                   # token tiles
    KC = n // 128                   # key chunks
    SG = 2                          # key chunks per S/E tile
    KC2 = KC // SG                  # E-tiles per (h,qc)
    QC = n // qch                   # query chunks
    DC = dim // 128                 # contraction chunks of dim
    MQK = 4                         # qk m-tiles: [q01, q23, k01, k23]
    scale = dh ** -0.5

    # ---------------- constants ----------------
    # 3 DMA-capable queues; per-queue order matters: tiles needed by the
    # first projection matmuls go first on each queue.
    dq = [nc.sync, nc.scalar, nc.gpsimd, nc.sync]
    cpool = ctx.enter_context(tc.tile_pool(name="consts", bufs=1))
    bqk = []
    for m in range(MQK):
        t = cpool.tile([128, 1], FP32, tag=f"bqk{m}", name=f"bqk{m}")
        nc.sync.dma_start(t[:], io["bqk"][m * 128:(m + 1) * 128, :])
        bqk.append(t)
    bv_row = cpool.tile([1, inner], FP16, tag="bv", name="bv_row")
    nc.sync.dma_start(bv_row[:], io["bv"][:])
    wqk_h = []
    wqk_x = []
    wv_sb = []
    for c in range(DC):
        t = cpool.tile([128, 2 * inner], FP16, tag=f"wqkh{c}", name=f"wqkh{c}")
        dq[c].dma_start(t[:], io["wqk_h"][c * 128:(c + 1) * 128, :])
        wqk_h.append(t)
    for c in range(DC):
        t = cpool.tile([128, 2 * inner], FP16, tag=f"wqkx{c}", name=f"wqkx{c}")
        dq[c].dma_start(t[:], io["wqk_x"][c * 128:(c + 1) * 128, :])
        wqk_x.append(t)
    for c in range(DC):
        t = cpool.tile([128, inner], FP16, tag=f"wv{c}", name=f"wv{c}")
        wv_sb.append(t)
    wout = []
    for m in range(2):
        t = cpool.tile([128, dim], BF16, tag=f"wout{m}", name=f"wout{m}")
        wout.append(t)
    ones_row_f16 = cpool.tile([1, 128], FP16, tag="ones_row16", name="ones_row16")
    nc.vector.memset(ones_row_f16[:], 1.0)
    # all-ones square: one matmul against the 0.01*acc limb pair both sums
    # over the chunk partitions and broadcasts c = 0.01*Z to all 128 rows
    ones128 = cpool.tile([128, 128], FP16, tag="ones128", name="ones128")
    nc.vector.memset(ones128[:], 1.0)

    # persistent activations
    apool = ctx.enter_context(tc.tile_pool(name="acts", bufs=1))
    # qhiT[m]: q hi-limb at natural rows (head 2m at 0:64, 2m+1 at 64:128)
    qhiT = [apool.tile([128, n], FP16, tag=f"qhiT{m}", name=f"qhiT{m}")
            for m in range(2)]
    # qh2[m]: qh mirrored to the opposite half (head 2m at 64:128, 2m+1 at
    # 0:64) so the hi*hi matmuls of adjacent key chunks can row-tile-pair.
    qh2 = [apool.tile([128, n], FP16, tag=f"qh2{m}", name=f"qh2{m}")
           for m in range(2)]
    # kh_alt[h]: kh of key chunk kc at row half (kc%2); other half unused.
    kh_alt = [apool.tile([128, n], FP16, tag=f"khA{h}", name=f"khA{h}")
              for h in range(hc)]
    # BQ[h]: rows rq(h) = ql, rows ro(h) = qh ; BK[h]: rows rq = kh, ro = kl
    BQ = [apool.tile([128, n], FP16, tag=f"BQ{h}", name=f"BQ{h}") for h in range(hc)]
    BK = [apool.tile([128, n], FP16, tag=f"BK{h}", name=f"BK{h}") for h in range(hc)]
    V_sb = [apool.tile([128, inner], BF16, tag=f"V{t}", name=f"V{t}")
            for t in range(NT)]
    attnB = [apool.tile([128, n], BF16, tag=f"attnB{m}", name=f"attnB{m}")
             for m in range(2)]

    # ---------------- phase B: projections ----------------
    nqs = 512
    with tc.tile_pool(name="xT", bufs=1) as xtp:
        xTh = []
        xTl = []
        for c in range(DC):
            xTh.append(xtp.tile([128, n], FP16, tag=f"xTh{c}", name=f"xTh{c}"))
            xTl.append(xtp.tile([128, n], FP16, tag=f"xTl{c}", name=f"xTl{c}"))
        # chunked loads so the first projection matmuls start early; queue c
        # carries dim-chunk c so the four chunks of a group land in parallel
        for nq in range(n // nqs):
            sl = slice(nq * nqs, (nq + 1) * nqs)
            for c in range(DC):
                dq[c].dma_start(xTh[c][:, sl],
                                io["xTh"][c * 128:(c + 1) * 128, sl])
                dq[c].dma_start(xTl[c][:, sl],
                                io["xTl"][c * 128:(c + 1) * 128, sl])
            if nq == 0:
                for c in range(DC):
                    dq[c].dma_start(wv_sb[c][:],
                                    io["wv"][c * 128:(c + 1) * 128, :])
            elif nq == 1:
                for m in range(2):
                    dq[m].dma_start(wout[m][:],
                                    io["wout_b"][m * 128:(m + 1) * 128, :])

        with tc.tile_pool(name="psB", bufs=4, space="PSUM") as psB:
            for m in range(MQK):
                msl = slice(m * 128, (m + 1) * 128)
                for nq in range(n // nqs):
                    sl = slice(nq * nqs, (nq + 1) * nqs)
                    ps = psB.tile([128, nqs], FP32, tag="psB")
                    for c in range(DC):
                        nc.tensor.matmul(ps[:], wqk_h[c][:, msl], xTh[c][:, sl],
                                         start=(c == 0), stop=False)
                    for c2 in range(2 * DC):
                        if c2 < DC:
                            lhsT, rhs = wqk_x[c2][:, msl], xTh[c2][:, sl]
                        else:
                            lhsT, rhs = wqk_h[c2 - DC][:, msl], xTl[c2 - DC][:, sl]
                        nc.tensor.matmul(ps[:], lhsT, rhs,
                                         start=False, stop=(c2 == 2 * DC - 1))
                    for hj in range(2):
                        rq, ro = 64 * hj, 64 * (1 - hj)
                        pss = ps[rq:rq + 64, :]
                        bsl = bqk[m][rq:rq + 64, :]
                        if m < 2:  # q m-tile, head h = 2m+hj
                            h = 2 * m + hj
                            hi = qhiT[m][rq:rq + 64, sl]
                            nc.vector.tensor_scalar(hi, pss, bsl, None, ALU.add)
                            nc.vector.tensor_copy(BQ[h][ro:ro + 64, sl], hi)
                            nc.vector.scalar_tensor_tensor(
                                BQ[h][rq:rq + 64, sl], pss, bsl, hi,
                                ALU.add, ALU.subtract)
                            nc.scalar.activation(qh2[m][ro:ro + 64, sl], hi,
                                                 AFT.Copy)
                        else:      # k m-tile, head h = 2(m-2)+hj
                            h = 2 * (m - 2) + hj
                            hi = BK[h][rq:rq + 64, sl]
                            nc.vector.tensor_scalar(hi, pss, bsl, None, ALU.add)
                            nc.vector.scalar_tensor_tensor(
                                BK[h][ro:ro + 64, sl], pss, bsl, hi,
                                ALU.add, ALU.subtract)
                            # kh mirrored for row-tile pairing: chunk kc at
                            # half (kc%2); nq*4 is even so local parity works
                            hi3 = hi.rearrange("p (k c) -> p k c", c=128)
                            ka = kh_alt[h]
                            nc.gpsimd.tensor_copy(
                                ka[0:64, sl].rearrange(
                                    "p (k c) -> p k c", c=128)[:, 0::2, :],
                                hi3[:, 0::2, :])
                            nc.gpsimd.tensor_copy(
                                ka[64:128, sl].rearrange(
                                    "p (k c) -> p k c", c=128)[:, 1::2, :],
                                hi3[:, 1::2, :])
            # V natural [n, inner] bf16, bias via rank-1 ones
            for nt in range(NT):
                ps = psB.tile([128, inner], FP32, tag="psBv")
                tsl = slice(nt * 128, (nt + 1) * 128)
                for c in range(DC):
                    nc.tensor.matmul(ps[:], xTh[c][:, tsl], wv_sb[c][:],
                                     start=(c == 0), stop=False)
                nc.tensor.matmul(ps[:], ones_row_f16[:], bv_row[:],
                                 start=False, stop=True)
                if nt % 2 == 0:
                    nc.vector.tensor_copy(V_sb[nt][:], ps[:])
                else:
                    nc.scalar.activation(V_sb[nt][:], ps[:], AFT.Copy)

    # ---------------- phase C: attention ----------------
    with tc.tile_pool(name="psS", bufs=2, space="PSUM") as psSp, \
         tc.tile_pool(name="psCB", bufs=2, space="PSUM") as psCBp, \
         tc.tile_pool(name="psO", bufs=1, space="PSUM") as psOp, \
         tc.tile_pool(name="psZR", bufs=1, space="PSUM") as psZRp, \
         tc.tile_pool(name="Epool", bufs=2 * KC2 - 1, space="SBUF") as Ep, \
         tc.tile_pool(name="accp", bufs=2) as accp, \
         tc.tile_pool(name="limb", bufs=2) as lp, \
         tc.tile_pool(name="mp", bufs=14, space="SBUF") as mp, \
         tc.tile_pool(name="crow", bufs=2) as crp:

        def stage_a(h, qc, bhead=None):
            """S^T matmuls, exp, Z tree accumulation for one (h, qc)."""
            mq, rq = h // 2, 64 * (h % 2)
            qsl = slice(qc * qch, (qc + 1) * qch)
            E_tiles = []
            acc = None
            # rows 0:64 hold qh_h in qt, rows 64:128 hold qh_h in qb
            qt = qhiT[mq] if h % 2 == 0 else qh2[mq]
            qb = qh2[mq] if h % 2 == 0 else qhiT[mq]
            for kt in range(KC2):
                ps = psSp.tile([128, SG * qch], FP32, tag="S")
                ksl0 = slice((SG * kt) * 128, (SG * kt + 1) * 128)
                ksl1 = slice((SG * kt + 1) * 128, (SG * kt + 2) * 128)
                # hi*hi of the two key chunks run concurrently (row-tiled at
                # partition halves 0/64)
                nc.tensor.matmul(ps[:, 0:qch], kh_alt[h][0:64, ksl0],
                                 qt[0:64, qsl], start=True, stop=False)
                nc.tensor.matmul(ps[:, qch:2 * qch], kh_alt[h][64:128, ksl1],
                                 qb[64:128, qsl], start=True, stop=False)
                nc.tensor.matmul(ps[:, 0:qch], BK[h][:, ksl0], BQ[h][:, qsl],
                                 start=False, stop=True)
                nc.tensor.matmul(ps[:, qch:2 * qch], BK[h][:, ksl1],
                                 BQ[h][:, qsl], start=False, stop=True)
                if kt == 1 and bhead is not None:
                    bhead()
                Et = Ep.tile([128, SG * qch], FP32, tag="E")
                nc.scalar.activation(Et[:], ps[:], AFT.Exp, scale=scale)
                E_tiles.append(Et)
                if kt == 1:
                    acc = accp.tile([128, SG * qch], FP32, tag="acc")
                    eng = nc.vector if TREE_ENG[0] == "v" else nc.gpsimd
                    eng.tensor_tensor(acc[:], E_tiles[0][:], Et[:], ALU.add)
                elif kt >= 2:
                    e = TREE_ENG[kt - 1]
                    if e == "d":
                        nc.gpsimd.dma_start(acc[:], Et[:], accum_op=ALU.add)
                    else:
                        eng = nc.vector if e == "v" else nc.gpsimd
                        eng.tensor_tensor(acc[:], acc[:], Et[:], ALU.add)
            return E_tiles, acc

        def stage_b_head(state):
            """j-fold, 0.01*acc limbs, c-broadcast + r matmuls for one (h, qc).

            One matmul of all-ones against the fp16 limb pair of 0.01*acc2
            both reduces over the chunk partitions and broadcasts
            c = 0.01*Z to every row of psC, replacing the separate Z-row
            and threshold-broadcast matmuls.
            """
            E_tiles, acc = state
            acc2 = acc[:, 0:qch]
            nc.gpsimd.tensor_tensor(acc2, acc[:, 0:qch], acc[:, qch:2 * qch],
                                    ALU.add)
            Eh = lp.tile([128, qch], FP16, tag="Eh")
            nc.scalar.activation(Eh[:], acc2, AFT.Copy, scale=0.01)
            El = lp.tile([128, qch], FP16, tag="El")
            nc.vector.scalar_tensor_tensor(El[:], acc2, 0.01, Eh[:],
                                           ALU.mult, ALU.subtract)
            psC = psCBp.tile([128, qch], FP32, tag="CB")
            nc.tensor.matmul(psC[:], ones128[:], Eh[:], start=True, stop=False)
            nc.tensor.matmul(psC[:], ones128[:], El[:], start=False, stop=True)
            # r = 1/Z = 0.01/c
            r_row = crp.tile([1, qch], FP32, tag="rrow")
            nc.vector.reciprocal_approx_fast(out=r_row[:], in_=psC[0:1, :])
            rh = crp.tile([1, qch], FP16, tag="rh")
            nc.scalar.activation(rh[:], r_row[:], AFT.Copy, scale=0.01)
            psZR = psZRp.tile([128, qch], FP32, tag="ZR")
            nc.tensor.matmul(psZR[64:128, :], ones_row_f16[:, :64], rh[:],
                             start=True, stop=True)
            r64 = crp.tile([64, qch], FP32, tag="r64")
            nc.scalar.activation(r64[:], psZR[64:128, :], AFT.Copy)
            return psC, r64

        def stage_masks(h, qc, state, head):
            """threshold masks for one (h, qc) -> P tiles."""
            E_tiles, acc = state
            psC, r64 = head
            cb_b = psC[:].unsqueeze(1).broadcast_to((128, SG, qch))
            P_tiles = []
            for kt in range(KC2):
                Et = E_tiles[kt]
                Pt = mp.tile([128, SG * qch], BF16, tag="P")
                nc.vector._custom_dve(
                    MASK_OP,
                    out=Pt[:].rearrange("p (j q) -> p j q", j=SG),
                    in0=Et[:].rearrange("p (j q) -> p j q", j=SG),
                    in1=cb_b)
                P_tiles.append(Pt)
            return P_tiles, r64

        def stage_pv_pair(a, b):
            """col-tiled PV for a head pair: hA -> psO rows 0:64 (array cols
            0:64), hB -> rows 64:128 (cols 64:128); the two matmuls per key
            chunk run concurrently."""
            (hA, qc), PA, r64A = a
            (hB, qcB), PB, r64B = b
            mq = hA // 2
            qsl = slice(qc * qch, (qc + 1) * qch)
            if PV_PAIR:
                psO = psOp.tile([128, qch], FP32, tag="O")
                for kc in range(KC):
                    js = slice((kc % SG) * qch, (kc % SG + 1) * qch)
                    nc.tensor.matmul(psO[0:64, :],
                                     V_sb[kc][:, hA * dh:(hA + 1) * dh],
                                     PA[kc // SG][:, js],
                                     start=(kc == 0), stop=(kc == KC - 1))
                    nc.tensor.matmul(psO[64:128, :],
                                     V_sb[kc][:, hB * dh:(hB + 1) * dh],
                                     PB[kc // SG][:, js],
                                     start=(kc == 0), stop=(kc == KC - 1))
                nc.vector.tensor_tensor(attnB[mq][0:64, qsl], psO[0:64, :],
                                        r64A[:], ALU.mult)
                nc.vector.tensor_tensor(attnB[mq][64:128, qsl], psO[64:128, :],
                                        r64B[:], ALU.mult)
            else:
                for h, P, r64, rq in ((hA, PA, r64A, 0), (hB, PB, r64B, 64)):
                    psO = psOp.tile([64, qch], FP32, tag="O")
                    for kc in range(KC):
                        js = slice((kc % SG) * qch, (kc % SG + 1) * qch)
                        nc.tensor.matmul(psO[:],
                                         V_sb[kc][:, h * dh:(h + 1) * dh],
                                         P[kc // SG][:, js],
                                         start=(kc == 0), stop=(kc == KC - 1))
                    nc.vector.tensor_tensor(attnB[mq][rq:rq + 64, qsl], psO[:],
                                            r64[:], ALU.mult)

        # qc-major so the (even, odd) head pair of each qc is adjacent
        order = [(h, qc) for qc in range(QC) for h in range(hc)]
        prev = None
        pend = None
        head_box = {}
        for hq in order:
            pstate = prev[1] if prev is not None else None
            bhead = (lambda s=pstate: head_box.__setitem__("h", stage_b_head(s))) \
                if pstate is not None else None
            state = stage_a(hq[0], hq[1], bhead)
            if prev is not None:
                masked = (prev[0], *stage_masks(prev[0][0], prev[0][1],
                                                prev[1], head_box.pop("h")))
                if pend is None:
                    pend = masked
                else:
                    stage_pv_pair(pend, masked)
                    pend = None
            prev = (hq, state)
        head = stage_b_head(prev[1])
        masked = (prev[0], *stage_masks(prev[0][0], prev[0][1], prev[1], head))
        stage_pv_pair(pend, masked)

    # ---------------- phase E: output projection ----------------
    with tc.tile_pool(name="psE", bufs=4, space="PSUM") as psE, \
         tc.tile_pool(name="ostage", bufs=4) as osp:
        for nt in range(NT):
            ps = psE.tile([128, dim], FP32, tag="psE")
            tsl = slice(nt * 128, (nt + 1) * 128)
            for m in range(2):
                nc.tensor.matmul(ps[:], attnB[m][:, tsl], wout[m][:],
                                 start=(m == 0), stop=(m == 1))
            ot = osp.tile([128, dim], FP32, tag="ostage")
            if nt % 2 == 0:
                nc.vector.tensor_copy(ot[:], ps[:])
            else:
                nc.scalar.activation(ot[:], ps[:], AFT.Copy)
            dq[nt % 4].dma_start(io["out"][tsl, :], ot[:])


def build_program(n=2048, dim=512, hc=4, dh=64, qch=512):
    nc = bacc.Bacc(trn_type="TRN2", target_bir_lowering=False, debug=False)
    inner = hc * dh
    io = {}

    def din(name, shape, dt):
        io[name] = nc.dram_tensor(name, shape, dt, kind="ExternalInput").ap()

    din("xTh", [dim, n], FP16)
    din("xTl", [dim, n], FP16)
    din("wqk_h", [dim, 2 * inner], FP16)
    din("wqk_x", [dim, 2 * inner], FP16)
    din("wv", [dim, inner], FP16)
    din("bqk", [2 * inner, 1], FP32)
    din("bv", [1, inner], FP16)
    din("wout_b", [inner, dim], BF16)
    io["out"] = nc.dram_tensor("out", [n, dim], FP32, kind="ExternalOutput").ap()

    with tile.TileContext(nc) as tc:
        with ExitStack() as ctx:
            emit_core_kernel(ctx, tc, io, n=n, dim=dim, hc=hc, dh=dh, qch=qch)
    nc.compile()
    return nc


def make_core_inputs(x_b, Wq, Wk, Wv, bq, bk, bv, Wout_g, n=2048, dim=512,
                     hc=4, dh=64):
    f16 = np.float16
    inner = hc * dh
    xT = np.ascontiguousarray(x_b.T)
    xTh = xT.astype(f16)
    xTl = (xT - xTh.astype(np.float32)).astype(f16)
    wqk = np.concatenate([Wq, Wk], axis=1)              # [dim, 2*inner]
    wqk_hi = wqk.astype(f16)
    wqk_lo = (wqk - wqk_hi.astype(np.float32)).astype(f16)
    wqk_x = wqk_lo                                      # [dim, 2*inner]
    return {
        "xTh": xTh, "xTl": xTl,
        "wqk_h": wqk_hi, "wqk_x": wqk_x,
        "wv": Wv.astype(f16),
        "bqk": np.concatenate([bq, bk]).reshape(2 * inner, 1).astype(np.float32),
        "bv": bv.reshape(1, inner).astype(f16),
        "wout_b": Wout_g.astype(ml_dtypes.bfloat16),
    }


@functools.lru_cache(maxsize=1)
def _cached_program():
    return build_program()


def kernel(x, Wqkv, bqkv, Wout, bout):
    x = np.asarray(x, dtype=np.float32)
    Wqkv = np.asarray(Wqkv, dtype=np.float32)
    bqkv = np.asarray(bqkv, dtype=np.float32)
    Wout = np.asarray(Wout, dtype=np.float32)
    bout = np.asarray(bout, dtype=np.float32)

    b, n, dim = x.shape
    H, dh = 8, 64
    inner = H * dh
    hc = 4
    Wq, Wk, Wv = Wqkv[:, :inner], Wqkv[:, inner:2 * inner], Wqkv[:, 2 * inner:]
    bq, bk, bv = bqkv[:inner], bqkv[inner:2 * inner], bqkv[2 * inner:]

    in_maps = []
    for c in range(8):
        bb, g = c // 2, c % 2
        hsl = slice(g * hc * dh, (g + 1) * hc * dh)
        in_maps.append(make_core_inputs(
            x[bb], Wq[:, hsl], Wk[:, hsl], Wv[:, hsl],
            bq[hsl], bk[hsl], bv[hsl], Wout[hsl, :],
            n=n, dim=dim, hc=hc, dh=dh))

    nc = _cached_program()
    res = bass_utils.run_bass_kernel_spmd(nc, in_maps, core_ids=list(range(8)))
    global LAST_RESULTS
    LAST_RESULTS = res
    out = np.empty((b, n, dim), dtype=np.float32)
    for bb in range(b):
        out[bb] = res.results[2 * bb]["out"] + res.results[2 * bb + 1]["out"] \
            + bout
    return out



# revision 24
# speedup vs baseline: 1.0718x; 1.0718x over previous
"""Trainium2 Bass kernel for thresholded multi-head attention (v2).

Computes, for x:[b,n,dim] with b=4, n=2048, dim=512, heads=8, dh=64:
    qkv = x @ Wqkv + bqkv ; split q,k,v per head
    dots = q k^T / sqrt(dh) ; attn = softmax(dots)
    attn = where(attn > 0.01, attn, 0) ; out = attn @ v
    return out @ Wout + bout

Sharding over 8 NeuronCores: core c handles batch b = c//2 and head group
g = c%2 (4 of the 8 heads); host sums the two partial output projections
per batch and adds bout.

Numerics: the attention threshold sits within 8.9e-7 (relative) of the
closest entry, so attn must be exact to ~5e-7 near 0.01 or a flip blows the
error budget. S logits are computed from fp16 hi/lo limbs in TWO matmuls:
kh^T qh (64-contraction) plus a stacked [kh;kl]^T [ql;qh] (128-contraction)
covering both cross terms in one PE pass; Z is an exact fp32 elementwise
tree-sum of the eight E-tiles on DVE/GPSIMD, reduced across partitions via
hi/lo fp16 limb matmuls; the attn>0.01 compare is fp32-exact against
c = 0.01*Z via a one-pass custom DVE select. x is transposed and limb-split
host-side. Broadcast matmuls (threshold c, 1/Z) run as fp16 limbs at
1 cyc/row. The PE-side Z/threshold work doubles as queue filler that keeps
the tensor engine's HAM clock gate at full rate.
"""
import os
import sys
import functools

import numpy as np

for _p in ("/opt/trn_rl_repo", "/root/.axon_site", "/root/.axon_site/_ro/trn_rl_repo"):
    if os.path.isdir(_p) and _p not in sys.path:
        sys.path.append(_p)

import ml_dtypes
from contextlib import ExitStack

import concourse.bass as bass
import concourse.bacc as bacc
import concourse.mybir as mybir
import concourse.tile as tile
from concourse import bass_utils

FP32 = mybir.dt.float32
FP16 = mybir.dt.float16
BF16 = mybir.dt.bfloat16
ALU = mybir.AluOpType
AFT = mybir.ActivationFunctionType

# engine for each of the 7 tree adds (kt=1..7):
# v=vector tt, g=gpsimd tt, d=software-DGE DMA with accumulate (runs on the
# DMA engines, nearly free for the compute engines). kt=1 must be v or g.
TREE_ENG = "gvggvgg"
PV_PAIR = True


def _register_mask_op():
    """One-pass masked keep: out = in0 if in1 < in0 else 0."""
    from concourse.dve_spec import Spec, Src0, Src1, Zero, select
    from concourse import dve_ops as dops

    name = "MASK_KEEP_GT_ANT"
    for op in dops.OPS:
        if op.name == name:
            return op
    op = dops.DveOp(
        name,
        Spec(
            body=select(Src1 < Src0, Src0, Zero),
            reference=lambda in0, in1, s0, s1, imm2: np.where(
                in1 < in0, in0, 0.0).astype(np.float32),
        ),
        subdim=False,
        uops_sha={"v3": "d86f8416d0d7b042", "v4": "f70e64aee8639ca3"},
    )
    dops.OPS.append(op)
    dops._SUB_OPCODE_FOR_NAME[name] = dops._CUSTOM_DVE_ROW_BASE + len(dops.OPS) - 1
    dops.CUSTOM_DVE_SPECS[name] = op.spec
    return op


MASK_OP = _register_mask_op()


def emit_core_kernel(ctx, tc, io, n=2048, dim=512, hc=4, dh=64, qch=512):
    nc = tc.nc
    inner = hc * dh                 # 256
    NT = n // 128                   # token tiles
    KC = n // 128                   # key chunks
    SG = 2                          # key chunks per S/E tile
    KC2 = KC // SG                  # E-tiles per (h,qc)
    QC = n // qch                   # query chunks
    DC = dim // 128                 # contraction chunks of dim
    MQK = 4                         # qk m-tiles: [q01, q23, k01, k23]
    scale = dh ** -0.5

    # ---------------- constants ----------------
    # 3 DMA-capable queues; per-queue order matters: tiles needed by the
    # first projection matmuls go first on each queue.
    dq = [nc.sync, nc.scalar, nc.gpsimd, nc.scalar]
    cpool = ctx.enter_context(tc.tile_pool(name="consts", bufs=1))
    bqk = []
    for m in range(MQK):
        t = cpool.tile([128, 1], FP32, tag=f"bqk{m}", name=f"bqk{m}")
        nc.sync.dma_start(t[:], io["bqk"][m * 128:(m + 1) * 128, :])
        bqk.append(t)
    bv_row = cpool.tile([1, inner], FP16, tag="bv", name="bv_row")
    nc.sync.dma_start(bv_row[:], io["bv"][:])
    wqk_h = []
    wqk_x = []
    wv_sb = []
    for c in range(DC):
        t = cpool.tile([128, 2 * inner], FP16, tag=f"wqkh{c}", name=f"wqkh{c}")
        dq[c].dma_start(t[:], io["wqk_h"][c * 128:(c + 1) * 128, :])
        wqk_h.append(t)
    for c in range(DC):
        t = cpool.tile([128, 2 * inner], FP16, tag=f"wqkx{c}", name=f"wqkx{c}")
        wqk_x.append(t)
    for c in range(DC):
        t = cpool.tile([128, inner], FP16, tag=f"wv{c}", name=f"wv{c}")
        wv_sb.append(t)
    wout = []
    for m in range(2):
        t = cpool.tile([128, dim], BF16, tag=f"wout{m}", name=f"wout{m}")
        wout.append(t)
    ones_row_f16 = cpool.tile([1, 128], FP16, tag="ones_row16", name="ones_row16")
    nc.vector.memset(ones_row_f16[:], 1.0)
    # all-ones square: one matmul against the 0.01*acc limb pair both sums
    # over the chunk partitions and broadcasts c = 0.01*Z to all 128 rows
    ones128 = cpool.tile([128, 128], FP16, tag="ones128", name="ones128")
    nc.vector.memset(ones128[:], 1.0)

    # persistent activations
    apool = ctx.enter_context(tc.tile_pool(name="acts", bufs=1))
    # qhiT[m]: q hi-limb at natural rows (head 2m at 0:64, 2m+1 at 64:128)
    qhiT = [apool.tile([128, n], FP16, tag=f"qhiT{m}", name=f"qhiT{m}")
            for m in range(2)]
    # qh2[m]: qh mirrored to the opposite half (head 2m at 64:128, 2m+1 at
    # 0:64) so the hi*hi matmuls of adjacent key chunks can row-tile-pair.
    qh2 = [apool.tile([128, n], FP16, tag=f"qh2{m}", name=f"qh2{m}")
           for m in range(2)]
    # kh_alt[h]: kh of key chunk kc at row half (kc%2); other half unused.
    kh_alt = [apool.tile([128, n], FP16, tag=f"khA{h}", name=f"khA{h}")
              for h in range(hc)]
    # BQ[h]: rows rq(h) = ql, rows ro(h) = qh ; BK[h]: rows rq = kh, ro = kl
    BQ = [apool.tile([128, n], FP16, tag=f"BQ{h}", name=f"BQ{h}") for h in range(hc)]
    BK = [apool.tile([128, n], FP16, tag=f"BK{h}", name=f"BK{h}") for h in range(hc)]
    V_sb = [apool.tile([128, inner], BF16, tag=f"V{t}", name=f"V{t}")
            for t in range(NT)]
    attnB = [apool.tile([128, n], BF16, tag=f"attnB{m}", name=f"attnB{m}")
             for m in range(2)]

    # ---------------- phase B: projections ----------------
    nqs = 512
    with tc.tile_pool(name="xT", bufs=1) as xtp:
        xTh = []
        xTl = []
        for c in range(DC):
            xTh.append(xtp.tile([128, n], FP16, tag=f"xTh{c}", name=f"xTh{c}"))
            xTl.append(xtp.tile([128, n], FP16, tag=f"xTl{c}", name=f"xTl{c}"))
        # chunked loads so the first projection matmuls start early; queue c
        # carries dim-chunk c so the four chunks of a group land in parallel
        for nq in range(n // nqs):
            sl = slice(nq * nqs, (nq + 1) * nqs)
            for c in range(DC):
                dq[c].dma_start(xTh[c][:, sl],
                                io["xTh"][c * 128:(c + 1) * 128, sl])
                dq[c].dma_start(xTl[c][:, sl],
                                io["xTl"][c * 128:(c + 1) * 128, sl])
            if nq == 0:
                for c in range(DC):
                    dq[c].dma_start(wqk_x[c][:],
                                    io["wqk_x"][c * 128:(c + 1) * 128, :])
                for c in range(DC):
                    dq[c].dma_start(wv_sb[c][:],
                                    io["wv"][c * 128:(c + 1) * 128, :])
            elif nq == 1:
                for m in range(2):
                    dq[m].dma_start(wout[m][:],
                                    io["wout_b"][m * 128:(m + 1) * 128, :])

        with tc.tile_pool(name="psB", bufs=4, space="PSUM") as psB:
            for m in (0, 2, 1, 3):
                msl = slice(m * 128, (m + 1) * 128)
                for nq in range(n // nqs):
                    sl = slice(nq * nqs, (nq + 1) * nqs)
                    ps = psB.tile([128, nqs], FP32, tag="psB")
                    for c in range(DC):
                        nc.tensor.matmul(ps[:], wqk_h[c][:, msl], xTh[c][:, sl],
                                         start=(c == 0), stop=False)
                    for c2 in range(2 * DC):
                        if c2 < DC:
                            lhsT, rhs = wqk_x[c2][:, msl], xTh[c2][:, sl]
                        else:
                            lhsT, rhs = wqk_h[c2 - DC][:, msl], xTl[c2 - DC][:, sl]
                        nc.tensor.matmul(ps[:], lhsT, rhs,
                                         start=False, stop=(c2 == 2 * DC - 1))
                    for hj in range(2):
                        rq, ro = 64 * hj, 64 * (1 - hj)
                        pss = ps[rq:rq + 64, :]
                        bsl = bqk[m][rq:rq + 64, :]
                        if m < 2:  # q m-tile, head h = 2m+hj
                            h = 2 * m + hj
                            hi = qhiT[m][rq:rq + 64, sl]
                            nc.vector.tensor_scalar(hi, pss, bsl, None, ALU.add)
                            nc.vector.tensor_copy(BQ[h][ro:ro + 64, sl], hi)
                            nc.vector.scalar_tensor_tensor(
                                BQ[h][rq:rq + 64, sl], pss, bsl, hi,
                                ALU.add, ALU.subtract)
                            nc.scalar.activation(qh2[m][ro:ro + 64, sl], hi,
                                                 AFT.Copy)
                        else:      # k m-tile, head h = 2(m-2)+hj
                            h = 2 * (m - 2) + hj
                            hi = BK[h][rq:rq + 64, sl]
                            nc.vector.tensor_scalar(hi, pss, bsl, None, ALU.add)
                            nc.vector.scalar_tensor_tensor(
                                BK[h][ro:ro + 64, sl], pss, bsl, hi,
                                ALU.add, ALU.subtract)
                            # kh mirrored for row-tile pairing: chunk kc at
                            # half (kc%2); nq*4 is even so local parity works
                            hi3 = hi.rearrange("p (k c) -> p k c", c=128)
                            ka = kh_alt[h]
                            nc.scalar.activation(
                                ka[0:64, sl].rearrange(
                                    "p (k c) -> p k c", c=128)[:, 0::2, :],
                                hi3[:, 0::2, :], AFT.Copy)
                            nc.scalar.activation(
                                ka[64:128, sl].rearrange(
                                    "p (k c) -> p k c", c=128)[:, 1::2, :],
                                hi3[:, 1::2, :], AFT.Copy)
            # V natural [n, inner] bf16, bias via rank-1 ones
            for nt in range(NT):
                ps = psB.tile([128, inner], FP32, tag="psBv")
                tsl = slice(nt * 128, (nt + 1) * 128)
                for c in range(DC):
                    nc.tensor.matmul(ps[:], xTh[c][:, tsl], wv_sb[c][:],
                                     start=(c == 0), stop=False)
                nc.tensor.matmul(ps[:], ones_row_f16[:], bv_row[:],
                                 start=False, stop=True)
                if nt % 2 == 0:
                    nc.vector.tensor_copy(V_sb[nt][:], ps[:])
                else:
                    nc.scalar.activation(V_sb[nt][:], ps[:], AFT.Copy)

    # ---------------- phase C: attention ----------------
    with tc.tile_pool(name="psS", bufs=2, space="PSUM") as psSp, \
         tc.tile_pool(name="psCB", bufs=2, space="PSUM") as psCBp, \
         tc.tile_pool(name="psO", bufs=1, space="PSUM") as psOp, \
         tc.tile_pool(name="psZR", bufs=1, space="PSUM") as psZRp, \
         tc.tile_pool(name="Epool", bufs=2 * KC2 - 1, space="SBUF") as Ep, \
         tc.tile_pool(name="accp", bufs=2) as accp, \
         tc.tile_pool(name="limb", bufs=2) as lp, \
         tc.tile_pool(name="mp", bufs=14, space="SBUF") as mp, \
         tc.tile_pool(name="crow", bufs=2) as crp:

        def stage_a(h, qc, bhead=None):
            """S^T matmuls, exp, Z tree accumulation for one (h, qc)."""
            mq, rq = h // 2, 64 * (h % 2)
            qsl = slice(qc * qch, (qc + 1) * qch)
            E_tiles = []
            acc = None
            # rows 0:64 hold qh_h in qt, rows 64:128 hold qh_h in qb
            qt = qhiT[mq] if h % 2 == 0 else qh2[mq]
            qb = qh2[mq] if h % 2 == 0 else qhiT[mq]
            for kt in range(KC2):
                ps = psSp.tile([128, SG * qch], FP32, tag="S")
                ksl0 = slice((SG * kt) * 128, (SG * kt + 1) * 128)
                ksl1 = slice((SG * kt + 1) * 128, (SG * kt + 2) * 128)
                # hi*hi of the two key chunks run concurrently (row-tiled at
                # partition halves 0/64)
                nc.tensor.matmul(ps[:, 0:qch], kh_alt[h][0:64, ksl0],
                                 qt[0:64, qsl], start=True, stop=False)
                nc.tensor.matmul(ps[:, qch:2 * qch], kh_alt[h][64:128, ksl1],
                                 qb[64:128, qsl], start=True, stop=False)
                nc.tensor.matmul(ps[:, 0:qch], BK[h][:, ksl0], BQ[h][:, qsl],
                                 start=False, stop=True)
                nc.tensor.matmul(ps[:, qch:2 * qch], BK[h][:, ksl1],
                                 BQ[h][:, qsl], start=False, stop=True)
                if kt == 1 and bhead is not None:
                    bhead()
                Et = Ep.tile([128, SG * qch], FP32, tag="E")
                nc.scalar.activation(Et[:], ps[:], AFT.Exp, scale=scale)
                E_tiles.append(Et)
                if kt == 1:
                    acc = accp.tile([128, SG * qch], FP32, tag="acc")
                    eng = nc.vector if TREE_ENG[0] == "v" else nc.gpsimd
                    eng.tensor_tensor(acc[:], E_tiles[0][:], Et[:], ALU.add)
                elif kt >= 2:
                    e = TREE_ENG[kt - 1]
                    if e == "d":
                        nc.gpsimd.dma_start(acc[:], Et[:], accum_op=ALU.add)
                    else:
                        eng = nc.vector if e == "v" else nc.gpsimd
                        eng.tensor_tensor(acc[:], acc[:], Et[:], ALU.add)
            return E_tiles, acc

        def stage_b_head(state):
            """j-fold, 0.01*acc limbs, c-broadcast + r matmuls for one (h, qc).

            One matmul of all-ones against the fp16 limb pair of 0.01*acc2
            both reduces over the chunk partitions and broadcasts
            c = 0.01*Z to every row of psC, replacing the separate Z-row
            and threshold-broadcast matmuls.
            """
            E_tiles, acc = state
            acc2 = acc[:, 0:qch]
            nc.gpsimd.tensor_tensor(acc2, acc[:, 0:qch], acc[:, qch:2 * qch],
                                    ALU.add)
            Eh = lp.tile([128, qch], FP16, tag="Eh")
            nc.scalar.activation(Eh[:], acc2, AFT.Copy, scale=0.01)
            El = lp.tile([128, qch], FP16, tag="El")
            nc.vector.scalar_tensor_tensor(El[:], acc2, 0.01, Eh[:],
                                           ALU.mult, ALU.subtract)
            psC = psCBp.tile([128, qch], FP32, tag="CB")
            nc.tensor.matmul(psC[:], ones128[:], Eh[:], start=True, stop=False)
            nc.tensor.matmul(psC[:], ones128[:], El[:], start=False, stop=True)
            # r = 1/Z = 0.01/c
            r_row = crp.tile([1, qch], FP32, tag="rrow")
            nc.vector.reciprocal_approx_fast(out=r_row[:], in_=psC[0:1, :])
            rh = crp.tile([1, qch], FP16, tag="rh")
            nc.scalar.activation(rh[:], r_row[:], AFT.Copy, scale=0.01)
            psZR = psZRp.tile([128, qch], FP32, tag="ZR")
            nc.tensor.matmul(psZR[64:128, :], ones_row_f16[:, :64], rh[:],
                             start=True, stop=True)
            r64 = crp.tile([64, qch], FP32, tag="r64")
            nc.scalar.activation(r64[:], psZR[64:128, :], AFT.Copy)
            return psC, r64

        def stage_masks(h, qc, state, head):
            """threshold masks for one (h, qc) -> P tiles."""
            E_tiles, acc = state
            psC, r64 = head
            cb_b = psC[:].unsqueeze(1).broadcast_to((128, SG, qch))
            P_tiles = []
            for kt in range(KC2):
                Et = E_tiles[kt]
                Pt = mp.tile([128, SG * qch], BF16, tag="P")
                nc.vector._custom_dve(
                    MASK_OP,
                    out=Pt[:].rearrange("p (j q) -> p j q", j=SG),
                    in0=Et[:].rearrange("p (j q) -> p j q", j=SG),
                    in1=cb_b)
                P_tiles.append(Pt)
            return P_tiles, r64

        def stage_pv_pair(a, b):
            """col-tiled PV for a head pair: hA -> psO rows 0:64 (array cols
            0:64), hB -> rows 64:128 (cols 64:128); the two matmuls per key
            chunk run concurrently."""
            (hA, qc), PA, r64A = a
            (hB, qcB), PB, r64B = b
            mq = hA // 2
            qsl = slice(qc * qch, (qc + 1) * qch)
            if PV_PAIR:
                psO = psOp.tile([128, qch], FP32, tag="O")
                for kc in range(KC):
                    js = slice((kc % SG) * qch, (kc % SG + 1) * qch)
                    nc.tensor.matmul(psO[0:64, :],
                                     V_sb[kc][:, hA * dh:(hA + 1) * dh],
                                     PA[kc // SG][:, js],
                                     start=(kc == 0), stop=(kc == KC - 1))
                    nc.tensor.matmul(psO[64:128, :],
                                     V_sb[kc][:, hB * dh:(hB + 1) * dh],
                                     PB[kc // SG][:, js],
                                     start=(kc == 0), stop=(kc == KC - 1))
                nc.vector.tensor_tensor(attnB[mq][0:64, qsl], psO[0:64, :],
                                        r64A[:], ALU.mult)
                nc.vector.tensor_tensor(attnB[mq][64:128, qsl], psO[64:128, :],
                                        r64B[:], ALU.mult)
            else:
                for h, P, r64, rq in ((hA, PA, r64A, 0), (hB, PB, r64B, 64)):
                    psO = psOp.tile([64, qch], FP32, tag="O")
                    for kc in range(KC):
                        js = slice((kc % SG) * qch, (kc % SG + 1) * qch)
                        nc.tensor.matmul(psO[:],
                                         V_sb[kc][:, h * dh:(h + 1) * dh],
                                         P[kc // SG][:, js],
                                         start=(kc == 0), stop=(kc == KC - 1))
                    nc.vector.tensor_tensor(attnB[mq][rq:rq + 64, qsl], psO[:],
                                            r64[:], ALU.mult)

        # qc-major so the (even, odd) head pair of each qc is adjacent
        order = [(h, qc) for qc in range(QC) for h in range(hc)]
        prev = None
        pend = None
        head_box = {}
        for hq in order:
            pstate = prev[1] if prev is not None else None
            bhead = (lambda s=pstate: head_box.__setitem__("h", stage_b_head(s))) \
                if pstate is not None else None
            state = stage_a(hq[0], hq[1], bhead)
            if prev is not None:
                masked = (prev[0], *stage_masks(prev[0][0], prev[0][1],
                                                prev[1], head_box.pop("h")))
                if pend is None:
                    pend = masked
                else:
                    stage_pv_pair(pend, masked)
                    pend = None
            prev = (hq, state)
        head = stage_b_head(prev[1])
        masked = (prev[0], *stage_masks(prev[0][0], prev[0][1], prev[1], head))
        stage_pv_pair(pend, masked)

    # ---------------- phase E: output projection ----------------
    with tc.tile_pool(name="psE", bufs=4, space="PSUM") as psE, \
         tc.tile_pool(name="ostage", bufs=4) as osp:
        for nt in range(NT):
            ps = psE.tile([128, dim], FP32, tag="psE")
            tsl = slice(nt * 128, (nt + 1) * 128)
            for m in range(2):
                nc.tensor.matmul(ps[:], attnB[m][:, tsl], wout[m][:],
                                 start=(m == 0), stop=(m == 1))
            ot = osp.tile([128, dim], FP32, tag="ostage")
            if nt % 2 == 0:
                nc.vector.tensor_copy(ot[:], ps[:])
            else:
                nc.scalar.activation(ot[:], ps[:], AFT.Copy)
            dq[nt % 4].dma_start(io["out"][tsl, :], ot[:])


def build_program(n=2048, dim=512, hc=4, dh=64, qch=512):
    nc = bacc.Bacc(trn_type="TRN2", target_bir_lowering=False, debug=False)
    inner = hc * dh
    io = {}

    def din(name, shape, dt):
        io[name] = nc.dram_tensor(name, shape, dt, kind="ExternalInput").ap()

    din("xTh", [dim, n], FP16)
    din("xTl", [dim, n], FP16)
    din("wqk_h", [dim, 2 * inner], FP16)
    din("wqk_x", [dim, 2 * inner], FP16)
    din("wv", [dim, inner], FP16)
    din("bqk", [2 * inner, 1], FP32)
    din("bv", [1, inner], FP16)
    din("wout_b", [inner, dim], BF16)
    io["out"] = nc.dram_tensor("out", [n, dim], FP32, kind="ExternalOutput").ap()

    with tile.TileContext(nc) as tc:
        with ExitStack() as ctx:
            emit_core_kernel(ctx, tc, io, n=n, dim=dim, hc=hc, dh=dh, qch=qch)
    nc.compile()
    return nc


def make_core_inputs(x_b, Wq, Wk, Wv, bq, bk, bv, Wout_g, n=2048, dim=512,
                     hc=4, dh=64):
    f16 = np.float16
    inner = hc * dh
    xT = np.ascontiguousarray(x_b.T)
    xTh = xT.astype(f16)
    xTl = (xT - xTh.astype(np.float32)).astype(f16)
    wqk = np.concatenate([Wq, Wk], axis=1)              # [dim, 2*inner]
    wqk_hi = wqk.astype(f16)
    wqk_lo = (wqk - wqk_hi.astype(np.float32)).astype(f16)
    wqk_x = wqk_lo                                      # [dim, 2*inner]
    return {
        "xTh": xTh, "xTl": xTl,
        "wqk_h": wqk_hi, "wqk_x": wqk_x,
        "wv": Wv.astype(f16),
        "bqk": np.concatenate([bq, bk]).reshape(2 * inner, 1).astype(np.float32),
        "bv": bv.reshape(1, inner).astype(f16),
        "wout_b": Wout_g.astype(ml_dtypes.bfloat16),
    }


@functools.lru_cache(maxsize=1)
def _cached_program():
    return build_program()


def kernel(x, Wqkv, bqkv, Wout, bout):
    x = np.asarray(x, dtype=np.float32)
    Wqkv = np.asarray(Wqkv, dtype=np.float32)
    bqkv = np.asarray(bqkv, dtype=np.float32)
    Wout = np.asarray(Wout, dtype=np.float32)
    bout = np.asarray(bout, dtype=np.float32)

    b, n, dim = x.shape
    H, dh = 8, 64
    inner = H * dh
    hc = 4
    Wq, Wk, Wv = Wqkv[:, :inner], Wqkv[:, inner:2 * inner], Wqkv[:, 2 * inner:]
    bq, bk, bv = bqkv[:inner], bqkv[inner:2 * inner], bqkv[2 * inner:]

    in_maps = []
    for c in range(8):
        bb, g = c // 2, c % 2
        hsl = slice(g * hc * dh, (g + 1) * hc * dh)
        in_maps.append(make_core_inputs(
            x[bb], Wq[:, hsl], Wk[:, hsl], Wv[:, hsl],
            bq[hsl], bk[hsl], bv[hsl], Wout[hsl, :],
            n=n, dim=dim, hc=hc, dh=dh))

    nc = _cached_program()
    res = bass_utils.run_bass_kernel_spmd(nc, in_maps, core_ids=list(range(8)))
    global LAST_RESULTS
    LAST_RESULTS = res
    out = np.empty((b, n, dim), dtype=np.float32)
    for bb in range(b):
        out[bb] = res.results[2 * bb]["out"] + res.results[2 * bb + 1]["out"] \
            + bout
    return out



# revision 25
# speedup vs baseline: 1.0907x; 1.0176x over previous
"""Trainium2 Bass kernel for thresholded multi-head attention (v2).

Computes, for x:[b,n,dim] with b=4, n=2048, dim=512, heads=8, dh=64:
    qkv = x @ Wqkv + bqkv ; split q,k,v per head
    dots = q k^T / sqrt(dh) ; attn = softmax(dots)
    attn = where(attn > 0.01, attn, 0) ; out = attn @ v
    return out @ Wout + bout

Sharding over 8 NeuronCores: core c handles batch b = c//2 and head group
g = c%2 (4 of the 8 heads); host sums the two partial output projections
per batch and adds bout.

Numerics: the attention threshold sits within 8.9e-7 (relative) of the
closest entry, so attn must be exact to ~5e-7 near 0.01 or a flip blows the
error budget. S logits are computed from fp16 hi/lo limbs in TWO matmuls:
kh^T qh (64-contraction) plus a stacked [kh;kl]^T [ql;qh] (128-contraction)
covering both cross terms in one PE pass; Z is an exact fp32 elementwise
tree-sum of the eight E-tiles on DVE/GPSIMD, reduced across partitions via
hi/lo fp16 limb matmuls; the attn>0.01 compare is fp32-exact against
c = 0.01*Z via a one-pass custom DVE select. x is transposed and limb-split
host-side. Broadcast matmuls (threshold c, 1/Z) run as fp16 limbs at
1 cyc/row. The PE-side Z/threshold work doubles as queue filler that keeps
the tensor engine's HAM clock gate at full rate.
"""
import os
import sys
import functools

import numpy as np

for _p in ("/opt/trn_rl_repo", "/root/.axon_site", "/root/.axon_site/_ro/trn_rl_repo"):
    if os.path.isdir(_p) and _p not in sys.path:
        sys.path.append(_p)

import ml_dtypes
from contextlib import ExitStack

import concourse.bass as bass
import concourse.bacc as bacc
import concourse.mybir as mybir
import concourse.tile as tile
from concourse import bass_utils

FP32 = mybir.dt.float32
FP16 = mybir.dt.float16
BF16 = mybir.dt.bfloat16
ALU = mybir.AluOpType
AFT = mybir.ActivationFunctionType

# engine for each of the 7 tree adds (kt=1..7):
# v=vector tt, g=gpsimd tt, d=software-DGE DMA with accumulate (runs on the
# DMA engines, nearly free for the compute engines). kt=1 must be v or g.
TREE_ENG = "gggggvg"
PV_PAIR = True


def _register_mask_op():
    """One-pass masked keep: out = in0 if in1 < in0 else 0."""
    from concourse.dve_spec import Spec, Src0, Src1, Zero, select
    from concourse import dve_ops as dops

    name = "MASK_KEEP_GT_ANT"
    for op in dops.OPS:
        if op.name == name:
            return op
    op = dops.DveOp(
        name,
        Spec(
            body=select(Src1 < Src0, Src0, Zero),
            reference=lambda in0, in1, s0, s1, imm2: np.where(
                in1 < in0, in0, 0.0).astype(np.float32),
        ),
        subdim=False,
        uops_sha={"v3": "d86f8416d0d7b042", "v4": "f70e64aee8639ca3"},
    )
    dops.OPS.append(op)
    dops._SUB_OPCODE_FOR_NAME[name] = dops._CUSTOM_DVE_ROW_BASE + len(dops.OPS) - 1
    dops.CUSTOM_DVE_SPECS[name] = op.spec
    return op


MASK_OP = _register_mask_op()


def emit_core_kernel(ctx, tc, io, n=2048, dim=512, hc=4, dh=64, qch=512):
    nc = tc.nc
    inner = hc * dh                 # 256
    NT = n // 128                   # token tiles
    KC = n // 128                   # key chunks
    SG = 2                          # key chunks per S/E tile
    KC2 = KC // SG                  # E-tiles per (h,qc)
    QC = n // qch                   # query chunks
    DC = dim // 128                 # contraction chunks of dim
    MQK = 4                         # qk m-tiles: [q01, q23, k01, k23]
    scale = dh ** -0.5

    # ---------------- constants ----------------
    # 3 DMA-capable queues; per-queue order matters: tiles needed by the
    # first projection matmuls go first on each queue.
    dq = [nc.sync, nc.scalar, nc.gpsimd, nc.scalar]
    cpool = ctx.enter_context(tc.tile_pool(name="consts", bufs=1))
    bqk = []
    for m in range(MQK):
        t = cpool.tile([128, 1], FP32, tag=f"bqk{m}", name=f"bqk{m}")
        nc.sync.dma_start(t[:], io["bqk"][m * 128:(m + 1) * 128, :])
        bqk.append(t)
    bv_row = cpool.tile([1, inner], FP16, tag="bv", name="bv_row")
    nc.sync.dma_start(bv_row[:], io["bv"][:])
    wqk_h = []
    wqk_x = []
    wv_sb = []
    for c in range(DC):
        t = cpool.tile([128, 2 * inner], FP16, tag=f"wqkh{c}", name=f"wqkh{c}")
        dq[c].dma_start(t[:], io["wqk_h"][c * 128:(c + 1) * 128, :])
        wqk_h.append(t)
    for c in range(DC):
        t = cpool.tile([128, 2 * inner], FP16, tag=f"wqkx{c}", name=f"wqkx{c}")
        wqk_x.append(t)
    for c in range(DC):
        t = cpool.tile([128, inner], FP16, tag=f"wv{c}", name=f"wv{c}")
        wv_sb.append(t)
    wout = []
    for m in range(2):
        t = cpool.tile([128, dim], BF16, tag=f"wout{m}", name=f"wout{m}")
        wout.append(t)
    ones_row_f16 = cpool.tile([1, 128], FP16, tag="ones_row16", name="ones_row16")
    nc.vector.memset(ones_row_f16[:], 1.0)
    # all-ones square: one matmul against the 0.01*acc limb pair both sums
    # over the chunk partitions and broadcasts c = 0.01*Z to all 128 rows
    ones128 = cpool.tile([128, 128], FP16, tag="ones128", name="ones128")
    nc.vector.memset(ones128[:], 1.0)

    # persistent activations
    apool = ctx.enter_context(tc.tile_pool(name="acts", bufs=1))
    # qhiT[m]: q hi-limb at natural rows (head 2m at 0:64, 2m+1 at 64:128)
    qhiT = [apool.tile([128, n], FP16, tag=f"qhiT{m}", name=f"qhiT{m}")
            for m in range(2)]
    # qh2[m]: qh mirrored to the opposite half (head 2m at 64:128, 2m+1 at
    # 0:64) so the hi*hi matmuls of adjacent key chunks can row-tile-pair.
    qh2 = [apool.tile([128, n], FP16, tag=f"qh2{m}", name=f"qh2{m}")
           for m in range(2)]
    # kh_alt[h]: kh of key chunk kc at row half (kc%2); other half unused.
    kh_alt = [apool.tile([128, n], FP16, tag=f"khA{h}", name=f"khA{h}")
              for h in range(hc)]
    # BQ[h]: rows rq(h) = ql, rows ro(h) = qh ; BK[h]: rows rq = kh, ro = kl
    BQ = [apool.tile([128, n], FP16, tag=f"BQ{h}", name=f"BQ{h}") for h in range(hc)]
    BK = [apool.tile([128, n], FP16, tag=f"BK{h}", name=f"BK{h}") for h in range(hc)]
    V_sb = [apool.tile([128, inner], BF16, tag=f"V{t}", name=f"V{t}")
            for t in range(NT)]
    attnB = [apool.tile([128, n], BF16, tag=f"attnB{m}", name=f"attnB{m}")
             for m in range(2)]

    # ---------------- phase B: projections ----------------
    nqs = 512
    with tc.tile_pool(name="xT", bufs=1) as xtp:
        xTh = []
        xTl = []
        for c in range(DC):
            xTh.append(xtp.tile([128, n], FP16, tag=f"xTh{c}", name=f"xTh{c}"))
            xTl.append(xtp.tile([128, n], FP16, tag=f"xTl{c}", name=f"xTl{c}"))
        # chunked loads so the first projection matmuls start early; queue c
        # carries dim-chunk c so the four chunks of a group land in parallel
        for nq in range(n // nqs):
            sl = slice(nq * nqs, (nq + 1) * nqs)
            for c in range(DC):
                dq[c].dma_start(xTh[c][:, sl],
                                io["xTh"][c * 128:(c + 1) * 128, sl])
                dq[c].dma_start(xTl[c][:, sl],
                                io["xTl"][c * 128:(c + 1) * 128, sl])
            if nq == 0:
                for c in range(DC):
                    dq[c].dma_start(wqk_x[c][:],
                                    io["wqk_x"][c * 128:(c + 1) * 128, :])
                for c in range(DC):
                    dq[c].dma_start(wv_sb[c][:],
                                    io["wv"][c * 128:(c + 1) * 128, :])
            elif nq == 1:
                for m in range(2):
                    dq[m].dma_start(wout[m][:],
                                    io["wout_b"][m * 128:(m + 1) * 128, :])

        with tc.tile_pool(name="psB", bufs=4, space="PSUM") as psB:
            for m in (0, 2, 1, 3):
                msl = slice(m * 128, (m + 1) * 128)
                for nq in range(n // nqs):
                    sl = slice(nq * nqs, (nq + 1) * nqs)
                    ps = psB.tile([128, nqs], FP32, tag="psB")
                    for c in range(DC):
                        nc.tensor.matmul(ps[:], wqk_h[c][:, msl], xTh[c][:, sl],
                                         start=(c == 0), stop=False)
                    for c2 in range(2 * DC):
                        if c2 < DC:
                            lhsT, rhs = wqk_x[c2][:, msl], xTh[c2][:, sl]
                        else:
                            lhsT, rhs = wqk_h[c2 - DC][:, msl], xTl[c2 - DC][:, sl]
                        nc.tensor.matmul(ps[:], lhsT, rhs,
                                         start=False, stop=(c2 == 2 * DC - 1))
                    for hj in range(2):
                        rq, ro = 64 * hj, 64 * (1 - hj)
                        pss = ps[rq:rq + 64, :]
                        bsl = bqk[m][rq:rq + 64, :]
                        if m < 2:  # q m-tile, head h = 2m+hj
                            h = 2 * m + hj
                            hi = qhiT[m][rq:rq + 64, sl]
                            nc.vector.tensor_scalar(hi, pss, bsl, None, ALU.add)
                            nc.vector.tensor_copy(BQ[h][ro:ro + 64, sl], hi)
                            nc.vector.scalar_tensor_tensor(
                                BQ[h][rq:rq + 64, sl], pss, bsl, hi,
                                ALU.add, ALU.subtract)
                            nc.scalar.activation(qh2[m][ro:ro + 64, sl], hi,
                                                 AFT.Copy)
                        else:      # k m-tile, head h = 2(m-2)+hj
                            h = 2 * (m - 2) + hj
                            hi = BK[h][rq:rq + 64, sl]
                            nc.vector.tensor_scalar(hi, pss, bsl, None, ALU.add)
                            nc.vector.scalar_tensor_tensor(
                                BK[h][ro:ro + 64, sl], pss, bsl, hi,
                                ALU.add, ALU.subtract)
                            # kh mirrored for row-tile pairing: chunk kc at
                            # half (kc%2); nq*4 is even so local parity works
                            hi3 = hi.rearrange("p (k c) -> p k c", c=128)
                            ka = kh_alt[h]
                            nc.scalar.activation(
                                ka[0:64, sl].rearrange(
                                    "p (k c) -> p k c", c=128)[:, 0::2, :],
                                hi3[:, 0::2, :], AFT.Copy)
                            nc.scalar.activation(
                                ka[64:128, sl].rearrange(
                                    "p (k c) -> p k c", c=128)[:, 1::2, :],
                                hi3[:, 1::2, :], AFT.Copy)
            # V natural [n, inner] bf16, bias via rank-1 ones
            for nt in range(NT):
                ps = psB.tile([128, inner], FP32, tag="psBv")
                tsl = slice(nt * 128, (nt + 1) * 128)
                for c in range(DC):
                    nc.tensor.matmul(ps[:], xTh[c][:, tsl], wv_sb[c][:],
                                     start=(c == 0), stop=False)
                nc.tensor.matmul(ps[:], ones_row_f16[:], bv_row[:],
                                 start=False, stop=True)
                if nt % 2 == 0:
                    nc.vector.tensor_copy(V_sb[nt][:], ps[:])
                else:
                    nc.scalar.activation(V_sb[nt][:], ps[:], AFT.Copy)

    # ---------------- phase C: attention ----------------
    with tc.tile_pool(name="psS", bufs=2, space="PSUM") as psSp, \
         tc.tile_pool(name="psCB", bufs=2, space="PSUM") as psCBp, \
         tc.tile_pool(name="psO", bufs=1, space="PSUM") as psOp, \
         tc.tile_pool(name="psZR", bufs=1, space="PSUM") as psZRp, \
         tc.tile_pool(name="Epool", bufs=2 * KC2 - 1, space="SBUF") as Ep, \
         tc.tile_pool(name="accp", bufs=2) as accp, \
         tc.tile_pool(name="limb", bufs=2) as lp, \
         tc.tile_pool(name="mp", bufs=14, space="SBUF") as mp, \
         tc.tile_pool(name="crow", bufs=2) as crp:

        def stage_a(h, qc, bhead=None, bmask=None):
            """S^T matmuls, exp, Z tree accumulation for one (h, qc)."""
            mq, rq = h // 2, 64 * (h % 2)
            qsl = slice(qc * qch, (qc + 1) * qch)
            E_tiles = []
            acc = None
            # rows 0:64 hold qh_h in qt, rows 64:128 hold qh_h in qb
            qt = qhiT[mq] if h % 2 == 0 else qh2[mq]
            qb = qh2[mq] if h % 2 == 0 else qhiT[mq]
            for kt in range(KC2):
                ps = psSp.tile([128, SG * qch], FP32, tag="S")
                ksl0 = slice((SG * kt) * 128, (SG * kt + 1) * 128)
                ksl1 = slice((SG * kt + 1) * 128, (SG * kt + 2) * 128)
                # hi*hi of the two key chunks run concurrently (row-tiled at
                # partition halves 0/64)
                nc.tensor.matmul(ps[:, 0:qch], kh_alt[h][0:64, ksl0],
                                 qt[0:64, qsl], start=True, stop=False)
                nc.tensor.matmul(ps[:, qch:2 * qch], kh_alt[h][64:128, ksl1],
                                 qb[64:128, qsl], start=True, stop=False)
                nc.tensor.matmul(ps[:, 0:qch], BK[h][:, ksl0], BQ[h][:, qsl],
                                 start=False, stop=True)
                nc.tensor.matmul(ps[:, qch:2 * qch], BK[h][:, ksl1],
                                 BQ[h][:, qsl], start=False, stop=True)
                if kt == 1 and bhead is not None:
                    bhead()
                if kt == 3 and bmask is not None:
                    bmask()
                Et = Ep.tile([128, SG * qch], FP32, tag="E")
                nc.scalar.activation(Et[:], ps[:], AFT.Exp, scale=scale)
                E_tiles.append(Et)
                if kt == 1:
                    acc = accp.tile([128, SG * qch], FP32, tag="acc")
                    eng = nc.vector if TREE_ENG[0] == "v" else nc.gpsimd
                    eng.tensor_tensor(acc[:], E_tiles[0][:], Et[:], ALU.add)
                elif kt >= 2:
                    e = TREE_ENG[kt - 1]
                    if e == "d":
                        nc.gpsimd.dma_start(acc[:], Et[:], accum_op=ALU.add)
                    else:
                        eng = nc.vector if e == "v" else nc.gpsimd
                        eng.tensor_tensor(acc[:], acc[:], Et[:], ALU.add)
            return E_tiles, acc

        def stage_b_head(state):
            """j-fold, 0.01*acc limbs, c-broadcast + r matmuls for one (h, qc).

            One matmul of all-ones against the fp16 limb pair of 0.01*acc2
            both reduces over the chunk partitions and broadcasts
            c = 0.01*Z to every row of psC, replacing the separate Z-row
            and threshold-broadcast matmuls.
            """
            E_tiles, acc = state
            acc2 = acc[:, 0:qch]
            nc.gpsimd.tensor_tensor(acc2, acc[:, 0:qch], acc[:, qch:2 * qch],
                                    ALU.add)
            Eh = lp.tile([128, qch], FP16, tag="Eh")
            nc.scalar.activation(Eh[:], acc2, AFT.Copy, scale=0.01)
            El = lp.tile([128, qch], FP16, tag="El")
            nc.vector.scalar_tensor_tensor(El[:], acc2, 0.01, Eh[:],
                                           ALU.mult, ALU.subtract)
            psC = psCBp.tile([128, qch], FP32, tag="CB")
            nc.tensor.matmul(psC[:], ones128[:], Eh[:], start=True, stop=False)
            nc.tensor.matmul(psC[:], ones128[:], El[:], start=False, stop=True)
            # r = 1/Z = 0.01/c
            r_row = crp.tile([1, qch], FP32, tag="rrow")
            nc.vector.reciprocal_approx_fast(out=r_row[:], in_=psC[0:1, :])
            rh = crp.tile([1, qch], FP16, tag="rh")
            nc.scalar.activation(rh[:], r_row[:], AFT.Copy, scale=0.01)
            psZR = psZRp.tile([128, qch], FP32, tag="ZR")
            nc.tensor.matmul(psZR[64:128, :], ones_row_f16[:, :64], rh[:],
                             start=True, stop=True)
            r64 = crp.tile([64, qch], FP32, tag="r64")
            nc.scalar.activation(r64[:], psZR[64:128, :], AFT.Copy)
            return psC, r64

        def stage_masks(h, qc, state, head):
            """threshold masks for one (h, qc) -> P tiles."""
            E_tiles, acc = state
            psC, r64 = head
            cb_b = psC[:].unsqueeze(1).broadcast_to((128, SG, qch))
            P_tiles = []
            for kt in range(KC2):
                Et = E_tiles[kt]
                Pt = mp.tile([128, SG * qch], BF16, tag="P")
                nc.vector._custom_dve(
                    MASK_OP,
                    out=Pt[:].rearrange("p (j q) -> p j q", j=SG),
                    in0=Et[:].rearrange("p (j q) -> p j q", j=SG),
                    in1=cb_b)
                P_tiles.append(Pt)
            return P_tiles, r64

        def stage_pv_pair(a, b):
            """col-tiled PV for a head pair: hA -> psO rows 0:64 (array cols
            0:64), hB -> rows 64:128 (cols 64:128); the two matmuls per key
            chunk run concurrently."""
            (hA, qc), PA, r64A = a
            (hB, qcB), PB, r64B = b
            mq = hA // 2
            qsl = slice(qc * qch, (qc + 1) * qch)
            if PV_PAIR:
                psO = psOp.tile([128, qch], FP32, tag="O")
                for kc in range(KC):
                    js = slice((kc % SG) * qch, (kc % SG + 1) * qch)
                    nc.tensor.matmul(psO[0:64, :],
                                     V_sb[kc][:, hA * dh:(hA + 1) * dh],
                                     PA[kc // SG][:, js],
                                     start=(kc == 0), stop=(kc == KC - 1))
                    nc.tensor.matmul(psO[64:128, :],
                                     V_sb[kc][:, hB * dh:(hB + 1) * dh],
                                     PB[kc // SG][:, js],
                                     start=(kc == 0), stop=(kc == KC - 1))
                nc.vector.tensor_tensor(attnB[mq][0:64, qsl], psO[0:64, :],
                                        r64A[:], ALU.mult)
                nc.vector.tensor_tensor(attnB[mq][64:128, qsl], psO[64:128, :],
                                        r64B[:], ALU.mult)
            else:
                for h, P, r64, rq in ((hA, PA, r64A, 0), (hB, PB, r64B, 64)):
                    psO = psOp.tile([64, qch], FP32, tag="O")
                    for kc in range(KC):
                        js = slice((kc % SG) * qch, (kc % SG + 1) * qch)
                        nc.tensor.matmul(psO[:],
                                         V_sb[kc][:, h * dh:(h + 1) * dh],
                                         P[kc // SG][:, js],
                                         start=(kc == 0), stop=(kc == KC - 1))
                    nc.vector.tensor_tensor(attnB[mq][rq:rq + 64, qsl], psO[:],
                                            r64[:], ALU.mult)

        # qc-major so the (even, odd) head pair of each qc is adjacent
        order = [(h, qc) for qc in range(QC) for h in range(hc)]
        prev = None
        pend = None
        head_box = {}
        mask_box = {}
        for hq in order:
            if prev is not None:
                ph, pqc = prev[0]
                pstate = prev[1]
                bhead = (lambda s=pstate:
                         head_box.__setitem__("h", stage_b_head(s)))
                bmask = (lambda h=ph, q=pqc, s=pstate:
                         mask_box.__setitem__("m", stage_masks(
                             h, q, s, head_box.pop("h"))))
            else:
                bhead = bmask = None
            state = stage_a(hq[0], hq[1], bhead, bmask)
            if prev is not None:
                masked = (prev[0], *mask_box.pop("m"))
                if pend is None:
                    pend = masked
                else:
                    stage_pv_pair(pend, masked)
                    pend = None
            prev = (hq, state)
        head = stage_b_head(prev[1])
        masked = (prev[0], *stage_masks(prev[0][0], prev[0][1], prev[1], head))
        stage_pv_pair(pend, masked)

    # ---------------- phase E: output projection ----------------
    with tc.tile_pool(name="psE", bufs=4, space="PSUM") as psE, \
         tc.tile_pool(name="ostage", bufs=4) as osp:
        for nt in range(NT):
            ps = psE.tile([128, dim], FP32, tag="psE")
            tsl = slice(nt * 128, (nt + 1) * 128)
            for m in range(2):
                nc.tensor.matmul(ps[:], attnB[m][:, tsl], wout[m][:],
                                 start=(m == 0), stop=(m == 1))
            ot = osp.tile([128, dim], FP32, tag="ostage")
            if nt % 2 == 0:
                nc.vector.tensor_copy(ot[:], ps[:])
            else:
                nc.scalar.activation(ot[:], ps[:], AFT.Copy)
            dq[nt % 4].dma_start(io["out"][tsl, :], ot[:])


def build_program(n=2048, dim=512, hc=4, dh=64, qch=512):
    nc = bacc.Bacc(trn_type="TRN2", target_bir_lowering=False, debug=False)
    inner = hc * dh
    io = {}

    def din(name, shape, dt):
        io[name] = nc.dram_tensor(name, shape, dt, kind="ExternalInput").ap()

    din("xTh", [dim, n], FP16)
    din("xTl", [dim, n], FP16)
    din("wqk_h", [dim, 2 * inner], FP16)
    din("wqk_x", [dim, 2 * inner], FP16)
    din("wv", [dim, inner], FP16)
    din("bqk", [2 * inner, 1], FP32)
    din("bv", [1, inner], FP16)
    din("wout_b", [inner, dim], BF16)
    io["out"] = nc.dram_tensor("out", [n, dim], FP32, kind="ExternalOutput").ap()

    with tile.TileContext(nc) as tc:
        with ExitStack() as ctx:
            emit_core_kernel(ctx, tc, io, n=n, dim=dim, hc=hc, dh=dh, qch=qch)
    nc.compile()
    return nc


def make_core_inputs(x_b, Wq, Wk, Wv, bq, bk, bv, Wout_g, n=2048, dim=512,
                     hc=4, dh=64):
    f16 = np.float16
    inner = hc * dh
    xT = np.ascontiguousarray(x_b.T)
    xTh = xT.astype(f16)
    xTl = (xT - xTh.astype(np.float32)).astype(f16)
    wqk = np.concatenate([Wq, Wk], axis=1)              # [dim, 2*inner]
    wqk_hi = wqk.astype(f16)
    wqk_lo = (wqk - wqk_hi.astype(np.float32)).astype(f16)
    wqk_x = wqk_lo                                      # [dim, 2*inner]
    return {
        "xTh": xTh, "xTl": xTl,
        "wqk_h": wqk_hi, "wqk_x": wqk_x,
        "wv": Wv.astype(f16),
        "bqk": np.concatenate([bq, bk]).reshape(2 * inner, 1).astype(np.float32),
        "bv": bv.reshape(1, inner).astype(f16),
        "wout_b": Wout_g.astype(ml_dtypes.bfloat16),
    }


@functools.lru_cache(maxsize=1)
def _cached_program():
    return build_program()


def kernel(x, Wqkv, bqkv, Wout, bout):
    x = np.asarray(x, dtype=np.float32)
    Wqkv = np.asarray(Wqkv, dtype=np.float32)
    bqkv = np.asarray(bqkv, dtype=np.float32)
    Wout = np.asarray(Wout, dtype=np.float32)
    bout = np.asarray(bout, dtype=np.float32)

    b, n, dim = x.shape
    H, dh = 8, 64
    inner = H * dh
    hc = 4
    Wq, Wk, Wv = Wqkv[:, :inner], Wqkv[:, inner:2 * inner], Wqkv[:, 2 * inner:]
    bq, bk, bv = bqkv[:inner], bqkv[inner:2 * inner], bqkv[2 * inner:]

    in_maps = []
    for c in range(8):
        bb, g = c // 2, c % 2
        hsl = slice(g * hc * dh, (g + 1) * hc * dh)
        in_maps.append(make_core_inputs(
            x[bb], Wq[:, hsl], Wk[:, hsl], Wv[:, hsl],
            bq[hsl], bk[hsl], bv[hsl], Wout[hsl, :],
            n=n, dim=dim, hc=hc, dh=dh))

    nc = _cached_program()
    res = bass_utils.run_bass_kernel_spmd(nc, in_maps, core_ids=list(range(8)))
    global LAST_RESULTS
    LAST_RESULTS = res
    out = np.empty((b, n, dim), dtype=np.float32)
    for bb in range(b):
        out[bb] = res.results[2 * bb]["out"] + res.results[2 * bb + 1]["out"] \
            + bout
    return out



# revision 29
# speedup vs baseline: 1.1265x; 1.0328x over previous
"""Trainium2 Bass kernel for thresholded multi-head attention (v2).

Computes, for x:[b,n,dim] with b=4, n=2048, dim=512, heads=8, dh=64:
    qkv = x @ Wqkv + bqkv ; split q,k,v per head
    dots = q k^T / sqrt(dh) ; attn = softmax(dots)
    attn = where(attn > 0.01, attn, 0) ; out = attn @ v
    return out @ Wout + bout

Sharding over 8 NeuronCores: core c handles batch b = c//2 and head group
g = c%2 (4 of the 8 heads); host sums the two partial output projections
per batch and adds bout.

Numerics: the attention threshold sits within 8.9e-7 (relative) of the
closest entry, so attn must be exact to ~5e-7 near 0.01 or a flip blows the
error budget. S logits are computed from fp16 hi/lo limbs in TWO matmuls:
kh^T qh (64-contraction) plus a stacked [kh;kl]^T [ql;qh] (128-contraction)
covering both cross terms in one PE pass; Z is an exact fp32 elementwise
tree-sum of the eight E-tiles on DVE/GPSIMD, reduced across partitions via
hi/lo fp16 limb matmuls; the attn>0.01 compare is fp32-exact against
c = 0.01*Z via a one-pass custom DVE select. x is transposed and limb-split
host-side. Broadcast matmuls (threshold c, 1/Z) run as fp16 limbs at
1 cyc/row. The PE-side Z/threshold work doubles as queue filler that keeps
the tensor engine's HAM clock gate at full rate.
"""
import os
import sys
import functools

import numpy as np

for _p in ("/opt/trn_rl_repo", "/root/.axon_site", "/root/.axon_site/_ro/trn_rl_repo"):
    if os.path.isdir(_p) and _p not in sys.path:
        sys.path.append(_p)

import ml_dtypes
from contextlib import ExitStack

import concourse.bass as bass
import concourse.bacc as bacc
import concourse.mybir as mybir
import concourse.tile as tile
from concourse import bass_utils

FP32 = mybir.dt.float32
FP16 = mybir.dt.float16
BF16 = mybir.dt.bfloat16
ALU = mybir.AluOpType
AFT = mybir.ActivationFunctionType

# engine for each of the 7 tree adds (kt=1..7):
# v=vector tt, g=gpsimd tt, d=software-DGE DMA with accumulate (runs on the
# DMA engines, nearly free for the compute engines). kt=1 must be v or g.
TREE_ENG = "gggggvg"
PV_PAIR = True


def _register_mask_op():
    """One-pass masked keep: out = in0 if in1 < in0 else 0."""
    from concourse.dve_spec import Spec, Src0, Src1, Zero, select
    from concourse import dve_ops as dops

    name = "MASK_KEEP_GT_ANT"
    for op in dops.OPS:
        if op.name == name:
            return op
    op = dops.DveOp(
        name,
        Spec(
            body=select(Src1 < Src0, Src0, Zero),
            reference=lambda in0, in1, s0, s1, imm2: np.where(
                in1 < in0, in0, 0.0).astype(np.float32),
        ),
        subdim=False,
        uops_sha={"v3": "d86f8416d0d7b042", "v4": "f70e64aee8639ca3"},
    )
    dops.OPS.append(op)
    dops._SUB_OPCODE_FOR_NAME[name] = dops._CUSTOM_DVE_ROW_BASE + len(dops.OPS) - 1
    dops.CUSTOM_DVE_SPECS[name] = op.spec
    return op


MASK_OP = _register_mask_op()


def emit_core_kernel(ctx, tc, io, n=2048, dim=512, hc=4, dh=64, qch=512):
    nc = tc.nc
    inner = hc * dh                 # 256
    NT = n // 128                   # token tiles
    KC = n // 128                   # key chunks
    SG = 2                          # key chunks per S/E tile
    KC2 = KC // SG                  # E-tiles per (h,qc)
    QC = n // qch                   # query chunks
    DC = dim // 128                 # contraction chunks of dim
    MQK = 4                         # qk m-tiles: [q01, q23, k01, k23]
    scale = dh ** -0.5

    # ---------------- constants ----------------
    # 3 DMA-capable queues; per-queue order matters: tiles needed by the
    # first projection matmuls go first on each queue.
    dq = [nc.sync, nc.scalar, nc.gpsimd, nc.scalar]
    cpool = ctx.enter_context(tc.tile_pool(name="consts", bufs=1))
    bqk = []
    for m in range(MQK):
        t = cpool.tile([128, 1], FP32, tag=f"bqk{m}", name=f"bqk{m}")
        nc.sync.dma_start(t[:], io["bqk"][m * 128:(m + 1) * 128, :])
        bqk.append(t)
    bv_row = cpool.tile([1, inner], FP16, tag="bv", name="bv_row")
    nc.sync.dma_start(bv_row[:], io["bv"][:])
    wqk_h = []
    wqk_x = []
    wv_sb = []
    for c in range(DC):
        t = cpool.tile([128, 2 * inner], FP16, tag=f"wqkh{c}", name=f"wqkh{c}")
        dq[c].dma_start(t[:], io["wqk_h"][c * 128:(c + 1) * 128, :])
        wqk_h.append(t)
    for c in range(DC):
        t = cpool.tile([128, 2 * inner], FP16, tag=f"wqkx{c}", name=f"wqkx{c}")
        wqk_x.append(t)
    for c in range(DC):
        t = cpool.tile([128, inner], FP16, tag=f"wv{c}", name=f"wv{c}")
        wv_sb.append(t)
    wout = []
    for m in range(2):
        t = cpool.tile([128, dim], BF16, tag=f"wout{m}", name=f"wout{m}")
        wout.append(t)
    ones_row_f16 = cpool.tile([1, 128], FP16, tag="ones_row16", name="ones_row16")
    nc.vector.memset(ones_row_f16[:], 1.0)
    # all-ones square: one matmul against the 0.01*acc limb pair both sums
    # over the chunk partitions and broadcasts c = 0.01*Z to all 128 rows
    ones128 = cpool.tile([128, 128], FP16, tag="ones128", name="ones128")
    nc.vector.memset(ones128[:], 1.0)

    # persistent activations
    apool = ctx.enter_context(tc.tile_pool(name="acts", bufs=1))
    # qhiT[m]: q hi-limb at natural rows (head 2m at 0:64, 2m+1 at 64:128)
    qhiT = [apool.tile([128, n], FP16, tag=f"qhiT{m}", name=f"qhiT{m}")
            for m in range(2)]
    # qh2[m]: qh mirrored to the opposite half (head 2m at 64:128, 2m+1 at
    # 0:64) so the hi*hi matmuls of adjacent key chunks can row-tile-pair.
    qh2 = [apool.tile([128, n], FP16, tag=f"qh2{m}", name=f"qh2{m}")
           for m in range(2)]
    # kh_alt[h]: kh of key chunk kc at row half (kc%2); other half unused.
    kh_alt = [apool.tile([128, n], FP16, tag=f"khA{h}", name=f"khA{h}")
              for h in range(hc)]
    # BQ[h]: rows rq(h) = ql, rows ro(h) = qh ; BK[h]: rows rq = kh, ro = kl
    BQ = [apool.tile([128, n], FP16, tag=f"BQ{h}", name=f"BQ{h}") for h in range(hc)]
    BK = [apool.tile([128, n], FP16, tag=f"BK{h}", name=f"BK{h}") for h in range(hc)]
    V_sb = [apool.tile([128, inner], BF16, tag=f"V{t}", name=f"V{t}")
            for t in range(NT)]
    attnB = [apool.tile([128, n], BF16, tag=f"attnB{m}", name=f"attnB{m}")
             for m in range(2)]

    # ---------------- phase B: projections ----------------
    nqs = 512
    with tc.tile_pool(name="xT", bufs=1) as xtp:
        xTh = []
        xTl = []
        for c in range(DC):
            xTh.append(xtp.tile([128, n], FP16, tag=f"xTh{c}", name=f"xTh{c}"))
            xTl.append(xtp.tile([128, n], FP16, tag=f"xTl{c}", name=f"xTl{c}"))
        # chunked loads so the first projection matmuls start early; queue c
        # carries dim-chunk c so the four chunks of a group land in parallel
        for nq in range(n // nqs):
            sl = slice(nq * nqs, (nq + 1) * nqs)
            for c in range(DC):
                dq[c].dma_start(xTh[c][:, sl],
                                io["xTh"][c * 128:(c + 1) * 128, sl])
                dq[c].dma_start(xTl[c][:, sl],
                                io["xTl"][c * 128:(c + 1) * 128, sl])
            if nq == 0:
                for c in range(DC):
                    dq[c].dma_start(wqk_x[c][:],
                                    io["wqk_x"][c * 128:(c + 1) * 128, :])
                for c in range(DC):
                    dq[c].dma_start(wv_sb[c][:],
                                    io["wv"][c * 128:(c + 1) * 128, :])
            elif nq == 1:
                for m in range(2):
                    dq[m].dma_start(wout[m][:],
                                    io["wout_b"][m * 128:(m + 1) * 128, :])

        with tc.tile_pool(name="psB", bufs=4, space="PSUM") as psB:
            for m in (0, 2, 1, 3):
                msl = slice(m * 128, (m + 1) * 128)
                for nq in range(n // nqs):
                    sl = slice(nq * nqs, (nq + 1) * nqs)
                    ps = psB.tile([128, nqs], FP32, tag="psB")
                    for c in range(DC):
                        nc.tensor.matmul(ps[:], wqk_h[c][:, msl], xTh[c][:, sl],
                                         start=(c == 0), stop=False)
                    for c2 in range(2 * DC):
                        if c2 < DC:
                            lhsT, rhs = wqk_x[c2][:, msl], xTh[c2][:, sl]
                        else:
                            lhsT, rhs = wqk_h[c2 - DC][:, msl], xTl[c2 - DC][:, sl]
                        nc.tensor.matmul(ps[:], lhsT, rhs,
                                         start=False, stop=(c2 == 2 * DC - 1))
                    for hj in range(2):
                        rq, ro = 64 * hj, 64 * (1 - hj)
                        pss = ps[rq:rq + 64, :]
                        bsl = bqk[m][rq:rq + 64, :]
                        if m < 2:  # q m-tile, head h = 2m+hj
                            h = 2 * m + hj
                            hi = qhiT[m][rq:rq + 64, sl]
                            nc.vector.tensor_scalar(hi, pss, bsl, None, ALU.add)
                            nc.vector.tensor_copy(BQ[h][ro:ro + 64, sl], hi)
                            nc.vector.scalar_tensor_tensor(
                                BQ[h][rq:rq + 64, sl], pss, bsl, hi,
                                ALU.add, ALU.subtract)
                            nc.scalar.activation(qh2[m][ro:ro + 64, sl], hi,
                                                 AFT.Copy)
                        else:      # k m-tile, head h = 2(m-2)+hj
                            h = 2 * (m - 2) + hj
                            hi = BK[h][rq:rq + 64, sl]
                            nc.vector.tensor_scalar(hi, pss, bsl, None, ALU.add)
                            nc.vector.scalar_tensor_tensor(
                                BK[h][ro:ro + 64, sl], pss, bsl, hi,
                                ALU.add, ALU.subtract)
                            # kh mirrored for row-tile pairing: chunk kc at
                            # half (kc%2); nq*4 is even so local parity works
                            hi3 = hi.rearrange("p (k c) -> p k c", c=128)
                            ka = kh_alt[h]
                            nc.scalar.activation(
                                ka[0:64, sl].rearrange(
                                    "p (k c) -> p k c", c=128)[:, 0::2, :],
                                hi3[:, 0::2, :], AFT.Copy)
                            nc.scalar.activation(
                                ka[64:128, sl].rearrange(
                                    "p (k c) -> p k c", c=128)[:, 1::2, :],
                                hi3[:, 1::2, :], AFT.Copy)
            # V natural [n, inner] bf16, bias via rank-1 ones
            for nt in range(NT):
                ps = psB.tile([128, inner], FP32, tag="psBv")
                tsl = slice(nt * 128, (nt + 1) * 128)
                for c in range(DC):
                    nc.tensor.matmul(ps[:], xTh[c][:, tsl], wv_sb[c][:],
                                     start=(c == 0), stop=False)
                nc.tensor.matmul(ps[:], ones_row_f16[:], bv_row[:],
                                 start=False, stop=True)
                if nt % 2 == 0:
                    nc.vector.tensor_copy(V_sb[nt][:], ps[:])
                else:
                    nc.scalar.activation(V_sb[nt][:], ps[:], AFT.Copy)

    # ---------------- phase C: attention ----------------
    with tc.tile_pool(name="psS", bufs=2, space="PSUM") as psSp, \
         tc.tile_pool(name="psCB", bufs=2, space="PSUM") as psCBp, \
         tc.tile_pool(name="psO", bufs=1, space="PSUM") as psOp, \
         tc.tile_pool(name="psZR", bufs=1, space="PSUM") as psZRp, \
         tc.tile_pool(name="Epool", bufs=2 * KC2 - 1, space="SBUF") as Ep, \
         tc.tile_pool(name="accp", bufs=2) as accp, \
         tc.tile_pool(name="limb", bufs=2) as lp, \
         tc.tile_pool(name="mp", bufs=14, space="SBUF") as mp, \
         tc.tile_pool(name="crow", bufs=2) as crp:

        def stage_a(h, qc, bhead=None, bmask=None):
            """S^T matmuls, exp, Z tree accumulation for one (h, qc)."""
            mq, rq = h // 2, 64 * (h % 2)
            qsl = slice(qc * qch, (qc + 1) * qch)
            E_tiles = []
            acc = None
            # rows 0:64 hold qh_h in qt, rows 64:128 hold qh_h in qb
            qt = qhiT[mq] if h % 2 == 0 else qh2[mq]
            qb = qh2[mq] if h % 2 == 0 else qhiT[mq]
            # Z sum as two parallel half-width chains: gpsimd accumulates the
            # even tiles' j-halves, vector the odd tiles'; short merge tail.
            acc_g = accp.tile([128, qch], FP32, tag="accg")
            acc_v = accp.tile([128, qch], FP32, tag="accv")
            for kt in range(KC2):
                ps = psSp.tile([128, SG * qch], FP32, tag="S")
                ksl0 = slice((SG * kt) * 128, (SG * kt + 1) * 128)
                ksl1 = slice((SG * kt + 1) * 128, (SG * kt + 2) * 128)
                # hi*hi of the two key chunks run concurrently (row-tiled at
                # partition halves 0/64)
                nc.tensor.matmul(ps[:, 0:qch], kh_alt[h][0:64, ksl0],
                                 qt[0:64, qsl], start=True, stop=False)
                nc.tensor.matmul(ps[:, qch:2 * qch], kh_alt[h][64:128, ksl1],
                                 qb[64:128, qsl], start=True, stop=False)
                nc.tensor.matmul(ps[:, 0:qch], BK[h][:, ksl0], BQ[h][:, qsl],
                                 start=False, stop=True)
                nc.tensor.matmul(ps[:, qch:2 * qch], BK[h][:, ksl1],
                                 BQ[h][:, qsl], start=False, stop=True)
                if kt == 1 and bhead is not None:
                    bhead()
                if kt == 3 and bmask is not None:
                    bmask(0)
                if kt == 5 and bmask is not None:
                    bmask(1)
                Et = Ep.tile([128, SG * qch], FP32, tag="E")
                nc.scalar.activation(Et[:], ps[:], AFT.Exp, scale=scale)
                E_tiles.append(Et)
                eng, acc = (nc.gpsimd, acc_g) if kt % 2 == 0 else \
                    (nc.vector, acc_v)
                if kt < 2:
                    eng.tensor_tensor(acc[:], Et[:, 0:qch], Et[:, qch:2 * qch],
                                      ALU.add)
                else:
                    eng.tensor_tensor(acc[:], acc[:], Et[:, 0:qch], ALU.add)
                    eng.tensor_tensor(acc[:], acc[:], Et[:, qch:2 * qch],
                                      ALU.add)
            nc.vector.tensor_tensor(acc_g[:], acc_g[:], acc_v[:], ALU.add)
            return E_tiles, acc_g

        def stage_b_head(state):
            """j-fold, 0.01*acc limbs, c-broadcast + r matmuls for one (h, qc).

            One matmul of all-ones against the fp16 limb pair of 0.01*acc2
            both reduces over the chunk partitions and broadcasts
            c = 0.01*Z to every row of psC, replacing the separate Z-row
            and threshold-broadcast matmuls.
            """
            E_tiles, acc = state
            Eh = lp.tile([128, qch], FP16, tag="Eh")
            nc.scalar.activation(Eh[:], acc[:], AFT.Copy, scale=0.01)
            El = lp.tile([128, qch], FP16, tag="El")
            nc.vector.scalar_tensor_tensor(El[:], acc[:], 0.01, Eh[:],
                                           ALU.mult, ALU.subtract)
            psC = psCBp.tile([128, qch], FP32, tag="CB")
            nc.tensor.matmul(psC[:], ones128[:], Eh[:], start=True, stop=False)
            nc.tensor.matmul(psC[:], ones128[:], El[:], start=False, stop=True)
            # r = 1/Z = 0.01/c
            r_row = crp.tile([1, qch], FP32, tag="rrow")
            nc.vector.reciprocal_approx_fast(out=r_row[:], in_=psC[0:1, :])
            rh = crp.tile([1, qch], FP16, tag="rh")
            nc.scalar.activation(rh[:], r_row[:], AFT.Copy, scale=0.01)
            psZR = psZRp.tile([128, qch], FP32, tag="ZR")
            nc.tensor.matmul(psZR[64:128, :], ones_row_f16[:, :64], rh[:],
                             start=True, stop=True)
            r64 = crp.tile([64, qch], FP32, tag="r64")
            nc.scalar.activation(r64[:], psZR[64:128, :], AFT.Copy)
            return psC, r64

        def stage_masks(state, head, part, P_tiles):
            """threshold masks for tiles [part*KC2/2, (part+1)*KC2/2)."""
            E_tiles, acc = state
            psC, r64 = head
            cb_b = psC[:].unsqueeze(1).broadcast_to((128, SG, qch))
            for kt in range(part * (KC2 // 2), (part + 1) * (KC2 // 2)):
                Et = E_tiles[kt]
                Pt = mp.tile([128, SG * qch], BF16, tag="P")
                nc.vector._custom_dve(
                    MASK_OP,
                    out=Pt[:].rearrange("p (j q) -> p j q", j=SG),
                    in0=Et[:].rearrange("p (j q) -> p j q", j=SG),
                    in1=cb_b)
                P_tiles.append(Pt)

        def stage_pv_pair(a, b):
            """col-tiled PV for a head pair: hA -> psO rows 0:64 (array cols
            0:64), hB -> rows 64:128 (cols 64:128); the two matmuls per key
            chunk run concurrently."""
            (hA, qc), PA, r64A = a
            (hB, qcB), PB, r64B = b
            mq = hA // 2
            qsl = slice(qc * qch, (qc + 1) * qch)
            if PV_PAIR:
                psO = psOp.tile([128, qch], FP32, tag="O")
                for kc in range(KC):
                    js = slice((kc % SG) * qch, (kc % SG + 1) * qch)
                    nc.tensor.matmul(psO[0:64, :],
                                     V_sb[kc][:, hA * dh:(hA + 1) * dh],
                                     PA[kc // SG][:, js],
                                     start=(kc == 0), stop=(kc == KC - 1))
                    nc.tensor.matmul(psO[64:128, :],
                                     V_sb[kc][:, hB * dh:(hB + 1) * dh],
                                     PB[kc // SG][:, js],
                                     start=(kc == 0), stop=(kc == KC - 1))
                nc.vector.tensor_tensor(attnB[mq][0:64, qsl], psO[0:64, :],
                                        r64A[:], ALU.mult)
                nc.vector.tensor_tensor(attnB[mq][64:128, qsl], psO[64:128, :],
                                        r64B[:], ALU.mult)
            else:
                for h, P, r64, rq in ((hA, PA, r64A, 0), (hB, PB, r64B, 64)):
                    psO = psOp.tile([64, qch], FP32, tag="O")
                    for kc in range(KC):
                        js = slice((kc % SG) * qch, (kc % SG + 1) * qch)
                        nc.tensor.matmul(psO[:],
                                         V_sb[kc][:, h * dh:(h + 1) * dh],
                                         P[kc // SG][:, js],
                                         start=(kc == 0), stop=(kc == KC - 1))
                    nc.vector.tensor_tensor(attnB[mq][rq:rq + 64, qsl], psO[:],
                                            r64[:], ALU.mult)

        # qc-major so the (even, odd) head pair of each qc is adjacent
        order = [(h, qc) for qc in range(QC) for h in range(hc)]
        prev = None
        pend = None
        head_box = {}
        for hq in order:
            if prev is not None:
                pstate = prev[1]
                P_list = []
                bhead = (lambda s=pstate:
                         head_box.__setitem__("h", stage_b_head(s)))
                bmask = (lambda part, s=pstate, P=P_list:
                         stage_masks(s, head_box["h"], part, P))
            else:
                bhead = bmask = None
            state = stage_a(hq[0], hq[1], bhead, bmask)
            if prev is not None:
                masked = (prev[0], P_list, head_box.pop("h")[1])
                if pend is None:
                    pend = masked
                else:
                    stage_pv_pair(pend, masked)
                    pend = None
            prev = (hq, state)
        head = stage_b_head(prev[1])
        P_list = []
        stage_masks(prev[1], head, 0, P_list)
        stage_masks(prev[1], head, 1, P_list)
        masked = (prev[0], P_list, head[1])
        stage_pv_pair(pend, masked)

    # ---------------- phase E: output projection ----------------
    with tc.tile_pool(name="psE", bufs=4, space="PSUM") as psE, \
         tc.tile_pool(name="ostage", bufs=4) as osp:
        for nt in range(NT):
            ps = psE.tile([128, dim], FP32, tag="psE")
            tsl = slice(nt * 128, (nt + 1) * 128)
            for m in range(2):
                nc.tensor.matmul(ps[:], attnB[m][:, tsl], wout[m][:],
                                 start=(m == 0), stop=(m == 1))
            ot = osp.tile([128, dim], FP32, tag="ostage")
            if nt % 2 == 0:
                nc.vector.tensor_copy(ot[:], ps[:])
            else:
                nc.scalar.activation(ot[:], ps[:], AFT.Copy)
            dq[nt % 4].dma_start(io["out"][tsl, :], ot[:])


def build_program(n=2048, dim=512, hc=4, dh=64, qch=512):
    nc = bacc.Bacc(trn_type="TRN2", target_bir_lowering=False, debug=False)
    inner = hc * dh
    io = {}

    def din(name, shape, dt):
        io[name] = nc.dram_tensor(name, shape, dt, kind="ExternalInput").ap()

    din("xTh", [dim, n], FP16)
    din("xTl", [dim, n], FP16)
    din("wqk_h", [dim, 2 * inner], FP16)
    din("wqk_x", [dim, 2 * inner], FP16)
    din("wv", [dim, inner], FP16)
    din("bqk", [2 * inner, 1], FP32)
    din("bv", [1, inner], FP16)
    din("wout_b", [inner, dim], BF16)
    io["out"] = nc.dram_tensor("out", [n, dim], FP32, kind="ExternalOutput").ap()

    with tile.TileContext(nc) as tc:
        with ExitStack() as ctx:
            emit_core_kernel(ctx, tc, io, n=n, dim=dim, hc=hc, dh=dh, qch=qch)
    nc.compile()
    return nc


def make_core_inputs(x_b, Wq, Wk, Wv, bq, bk, bv, Wout_g, n=2048, dim=512,
                     hc=4, dh=64):
    f16 = np.float16
    inner = hc * dh
    xT = np.ascontiguousarray(x_b.T)
    xTh = xT.astype(f16)
    xTl = (xT - xTh.astype(np.float32)).astype(f16)
    wqk = np.concatenate([Wq, Wk], axis=1)              # [dim, 2*inner]
    wqk_hi = wqk.astype(f16)
    wqk_lo = (wqk - wqk_hi.astype(np.float32)).astype(f16)
    wqk_x = wqk_lo                                      # [dim, 2*inner]
    return {
        "xTh": xTh, "xTl": xTl,
        "wqk_h": wqk_hi, "wqk_x": wqk_x,
        "wv": Wv.astype(f16),
        "bqk": np.concatenate([bq, bk]).reshape(2 * inner, 1).astype(np.float32),
        "bv": bv.reshape(1, inner).astype(f16),
        "wout_b": Wout_g.astype(ml_dtypes.bfloat16),
    }


@functools.lru_cache(maxsize=1)
def _cached_program():
    return build_program()


def kernel(x, Wqkv, bqkv, Wout, bout):
    x = np.asarray(x, dtype=np.float32)
    Wqkv = np.asarray(Wqkv, dtype=np.float32)
    bqkv = np.asarray(bqkv, dtype=np.float32)
    Wout = np.asarray(Wout, dtype=np.float32)
    bout = np.asarray(bout, dtype=np.float32)

    b, n, dim = x.shape
    H, dh = 8, 64
    inner = H * dh
    hc = 4
    Wq, Wk, Wv = Wqkv[:, :inner], Wqkv[:, inner:2 * inner], Wqkv[:, 2 * inner:]
    bq, bk, bv = bqkv[:inner], bqkv[inner:2 * inner], bqkv[2 * inner:]

    in_maps = []
    for c in range(8):
        bb, g = c // 2, c % 2
        hsl = slice(g * hc * dh, (g + 1) * hc * dh)
        in_maps.append(make_core_inputs(
            x[bb], Wq[:, hsl], Wk[:, hsl], Wv[:, hsl],
            bq[hsl], bk[hsl], bv[hsl], Wout[hsl, :],
            n=n, dim=dim, hc=hc, dh=dh))

    nc = _cached_program()
    res = bass_utils.run_bass_kernel_spmd(nc, in_maps, core_ids=list(range(8)))
    global LAST_RESULTS
    LAST_RESULTS = res
    out = np.empty((b, n, dim), dtype=np.float32)
    for bb in range(b):
        out[bb] = res.results[2 * bb]["out"] + res.results[2 * bb + 1]["out"] \
            + bout
    return out

